# revision 57
# baseline (speedup 1.0000x reference)
"""Development version of the full-device BiLSTM-CRF kernel. See design notes.

Layouts (per core, BL=32 sequences):
 - LSTM gate-major: partitions = [fwd feat 64; bwd feat 64]; psum free =
   (pair-parity, gate, batch32). Two 16-seq groups pipeline the step chain.
 - gx bulk-matmul'd (f32r/bf16, N=512) into DRAM per direction; identity
   matmul accumulates into PSUM per step pair.
 - Viterbi forward: cp sharded 4-way across partition groups; score/e/onehot
   histories time-folded [128, T/4 * 41] (partition group = t%4).
 - Backtrace: onehot chain via PE matmul with trans^T, TTR fused add+max.
"""
import sys
sys.path.insert(0, '/opt/trn_rl_repo')
import numpy as np
import ml_dtypes
import concourse.bass as bass
import concourse.mybir as mybir
from concourse.tile import TileContext

F32 = mybir.dt.float32
F32R = mybir.dt.float32  # f32r reverted: interp models f32r with reduced precision
BF16 = mybir.dt.float32  # precision experiment: all-f32
I32 = mybir.dt.int32
AF = mybir.ActivationFunctionType
OP = mybir.AluOpType
AX = mybir.AxisListType

B, D_IN, HID, C = 256, 39, 128, 41
H = HID // 2
G4 = 4 * H
NCORES = 8
BL = B // NCORES
CP = 44
NG = 4
CW = 11
NEG = -1.0e30


def legalize_waits(nc):
    n = 0
    for _, bbw in nc.bb_map.items():
        il = bbw.bb.instructions
        out = []
        for i in il:
            si = getattr(i, 'sync_info', None)
            ow = list(si.on_wait) if (si is not None and si.on_wait) else []
            if len(ow) > 1:
                for w in ow[:-1]:
                    n += 1
                    es = mybir.InstEventSemaphore(
                        name=f"legwait-{n}-{i.name}", engine=i.engine, ins=[], outs=[],
                        sync_info=mybir.SyncInfo(on_wait=[w], on_update=[]))
                    out.append(es)
                i.sync_info = mybir.SyncInfo(on_wait=[ow[-1]], on_update=list(si.on_update or []))
            out.append(i)
        bbw.bb.instructions = out
    return n


def prep_weights(w_ih_l0, w_hh_l0, b_l0, w_ih_r, w_hh_r, b_r,
                 lin_w, lin_b, crf_start, crf_end, crf_trans):
    """Gate order i,f,g,o. g rows scaled x2 (tanh(z) = 2*sigmoid(2z)-1)."""
    d = {}

    def gscale(m):
        m = np.asarray(m, np.float32).copy()
        m[2 * H:3 * H] *= 2.0
        return m

    for di, nm in ((0, 'f'), (1, 'b')):
        w = gscale(w_ih_l0[di])
        bb = gscale(b_l0[di])
        d[f'wx0_{nm}'] = np.concatenate([w.T, bb[None, :]], 0).astype(np.float32)
    for li in (0, 1):
        for di, nm in ((0, 'f'), (1, 'b')):
            w = gscale(w_ih_r[li, di])
            bb = gscale(b_r[li, di])
            d[f'wx{li+1}_{nm}'] = np.ascontiguousarray(w.T).astype(np.float32)
            d[f'bias{li+1}_{nm}'] = bb[None, :].astype(np.float32)
    for li in range(3):
        whh = np.asarray(w_hh_l0) if li == 0 else np.asarray(w_hh_r[li - 1])
        for gi in range(4):
            blk = np.zeros((128, 128), np.float32)
            sc = 2.0 if gi == 2 else 1.0
            blk[0:64, 0:64] = sc * whh[0, gi * H:(gi + 1) * H, :].T
            blk[64:128, 64:128] = sc * whh[1, gi * H:(gi + 1) * H, :].T
            d[f'whh{li}_{gi}'] = blk.astype(np.float32)
    d['ident128'] = np.eye(128, dtype=np.float32)
    d['ident16'] = np.eye(16, dtype=np.float32)
    d['ident32'] = np.eye(32, dtype=np.float32)
    d['ident44'] = np.eye(CP, dtype=np.float32)
    lw = np.zeros((HID, CP), np.float32)
    lw[:, :C] = np.asarray(lin_w, np.float32).T
    d['linWT'] = lw.astype(np.float32)
    lb = np.full((CP, 1), NEG, np.float32)
    lb[:C, 0] = np.asarray(lin_b, np.float32)
    d['linB'] = lb
    tr = np.asarray(crf_trans, np.float32)
    # transB_cn[p=(b,g), (ci, cp)] = trans[cp, g*CW+ci], NEG for pads
    transB = np.full((128, CW, CP), NEG, np.float32)
    for g in range(NG):
        for ci in range(CW):
            cn = g * CW + ci
            if cn < C:
                for b in range(32):
                    transB[b * 4 + g, ci, :C] = tr[:, cn]
    d['transB'] = transB.reshape(128, CW * CP)
    trT = np.full((C, CP), NEG, np.float32)
    trT[:, :C] = tr.T  # [cn, cp]
    d['transT'] = trT
    d['transThi'] = np.ascontiguousarray(trT[32:41])  # cn 32..40 rows
    st = np.full((128, CP), NEG, np.float32)
    st[:, :C] = np.asarray(crf_start, np.float32)[None, :]
    d['startRep'] = st
    en = np.full((32, CP), NEG, np.float32)
    en[:, :C] = np.asarray(crf_end, np.float32)[None, :]
    d['endRep'] = en
    io = np.zeros((128, CP), np.float32)
    io[:, :C] = np.arange(C, dtype=np.float32)[None, :]
    d['iotaRep'] = io
    d['onesrow'] = np.ones((1, 512), np.float32)
    gG = np.zeros((128, 4, 128), np.float32)
    for b in range(32):
        for g in range(4):
            for gp in range(4):
                gG[b * 4 + g, g, b * 4 + gp] = 1.0
    d['gatherG'] = gG.reshape(128, 512)
    d['zeros16'] = np.zeros((128, 16), np.float32)
    return d


def shard_x(x, cid, T):
    xs = np.asarray(x, np.float32)[cid * BL:(cid + 1) * BL, :T]
    xt = np.empty((D_IN + 1, T * BL), np.float32)
    xt[D_IN] = 1.0
    xt[:D_IN] = xs.transpose(2, 1, 0).reshape(D_IN, T * BL)
    return xt.astype(np.float32)


def build_nc(T):
    R = BL * T
    TJ = T // 4
    NCH = R // 512
    nc = bass.Bass()
    dt = {}

    def din(name, shape, dty=F32):
        dt[name] = nc.dram_tensor(name, shape, dty, kind="ExternalInput")

    din('xT', [D_IN + 1, R], F32R)
    din('wx0_f', [40, 256], F32R); din('wx0_b', [40, 256], F32R)
    for li in (1, 2):
        for nm in ('f', 'b'):
            din(f'wx{li}_{nm}', [128, 256], F32R)
            din(f'bias{li}_{nm}', [1, 256], F32R)
    for li in range(3):
        for gi in range(4):
            din(f'whh{li}_{gi}', [128, 128], F32)
    din('ident128', [128, 128], BF16); din('ident16', [16, 16]); din('ident32', [32, 32]); din('ident44', [CP, CP])
    din('linWT', [HID, CP], F32R); din('linB', [CP, 1])
    din('transB', [128, CW * CP]); din('transT', [C, CP])
    din('transThi', [9, CP])
    din('startRep', [128, CP]); din('endRep', [32, CP]); din('iotaRep', [128, CP])
    din('onesrow', [1, 512], F32R); din('zeros16', [128, 16], F32)
    din('gatherG', [128, 512])

    def scratch(name, shape, dty=F32):
        dt[name] = nc.dram_tensor(name, shape, dty, kind="Internal")

    for li3 in range(3):
        scratch(f'gx{li3}_f', [64, T * 128], BF16)
        scratch(f'gx{li3}_b', [64, T * 128], BF16)
    scratch('hbuf0', [HID, R], F32R)
    scratch('hbuf1', [HID, R], F32R)
    scratch('hbuf2', [HID, R], F32R)
    scratch('e_d', [T, 128, CP])
    scratch('score_d', [T, 128, CP])
    scratch('oh_d', [T, 32, CP])
    dt['tags'] = nc.dram_tensor('tags', [BL, T], I32, kind="ExternalOutput")

    with TileContext(nc) as tc:
        with tc.tile_pool(name="const", bufs=1) as cpool, \
             tc.tile_pool(name="wpool", bufs=1) as wpool, \
             tc.tile_pool(name="hist", bufs=1) as hpool, \
             tc.tile_pool(name="bulk_rhs", bufs=4) as rhspool, \
             tc.tile_pool(name="gx", bufs=8) as gxpool, \
             tc.tile_pool(name="psum", bufs=2, space="PSUM") as pspool, \
             tc.tile_pool(name="sig", bufs=8) as sigpool, \
             tc.tile_pool(name="hc", bufs=8) as hcpool, \
             tc.tile_pool(name="vit", bufs=4) as vitpool, \
             tc.tile_pool(name="emis", bufs=2) as epool:

            def load_const(nm, shape, dty=F32):
                t = cpool.tile(shape, dty, tag=nm)
                nc.sync.dma_start(t[:], dt[nm][:])
                return t

            ident128 = load_const('ident128', [128, 128], BF16)
            ident16 = load_const('ident16', [16, 16])
            ident32 = load_const('ident32', [32, 32])
            ident44 = load_const('ident44', [CP, CP])
            linWT = load_const('linWT', [HID, CP], F32R)
            linB = load_const('linB', [CP, 1])
            transB = load_const('transB', [128, CW * CP])
            transT = load_const('transT', [C, CP])
            transThi = load_const('transThi', [9, CP])
            startRep = load_const('startRep', [128, CP])
            endRep = load_const('endRep', [32, CP])
            iotaRep = load_const('iotaRep', [128, CP])
            gatherG = load_const('gatherG', [128, 512])
            whh = {}
            for li in range(3):
                for gi in range(4):
                    whh[(li, gi)] = load_const(f'whh{li}_{gi}', [128, 128], F32)
            onesrow = load_const('onesrow', [1, 512], F32R)
            zeros16 = load_const('zeros16', [128, 16], F32)

            scoreRep = hpool.tile([128, CP], F32, tag="scoreRep")

            # ---------- bulk gx (micro-op generator for interleaving) ----------
            bulk_state = {}

            def bulk_load_weights(li):
                wx = {}
                bias = {}
                for nm in ('f', 'b'):
                    wx[nm] = wpool.tile([40 if li == 0 else 128, 256], F32R,
                                        tag=f"wx{li}_{nm}", name=f"wx{li}{nm}")
                    nc.sync.dma_start(wx[nm][:], dt[f'wx{li}_{nm}'][:])
                    if li > 0:
                        bias[nm] = wpool.tile([1, 256], F32R, tag=f"bias{li}_{nm}", name=f"bias{li}{nm}")
                        nc.sync.dma_start(bias[nm][:], dt[f'bias{li}_{nm}'][:])
                bulk_state[li] = (wx, bias)

            def bulk_chunk_ops(li, src_dram, src_k, ch):
                """Yield micro-closures; caller drains them spread over time."""
                wx, bias = bulk_state[li]
                rhs = rhspool.tile([src_k, 512], F32R, tag=f"rhs{li}")
                yield ('l', lambda: nc.sync.dma_start(
                    rhs[:], src_dram[:, ch * 512:(ch + 1) * 512]))
                for nm in ('f', 'b'):
                    for pr in range(2):
                        ps = pspool.tile([128, 512], F32, tag="big", name="bps")
                        if li == 0:
                            yield ('h', lambda ps=ps, nm=nm, pr=pr: nc.tensor.matmul(
                                ps[:], wx[nm][:, pr * 128:(pr + 1) * 128],
                                rhs[:], start=True, stop=True))
                        else:
                            yield ('h', lambda ps=ps, nm=nm, pr=pr: nc.tensor.matmul(
                                ps[:], wx[nm][:, pr * 128:(pr + 1) * 128],
                                rhs[:], start=True, stop=False))
                            yield ('h', lambda ps=ps, nm=nm, pr=pr: nc.tensor.matmul(
                                ps[:], bias[nm][:, pr * 128:(pr + 1) * 128],
                                onesrow[:], start=False, stop=True))
                        stg = rhspool.tile([128, 512], BF16, tag="gxstg",
                                           name="gxstg")
                        if (ch + pr) % 2 == 0:
                            yield ('l', lambda ps=ps, stg=stg: nc.scalar.activation(
                                stg[:], ps[:], AF.Copy))
                        else:
                            yield ('l', lambda ps=ps, stg=stg: nc.vector.tensor_copy(
                                stg[:], ps[:]))
                        gxd = dt[f'gx{li}_{nm}']
                        t0c = ch * 16
                        for gl in range(2):
                            gi4 = pr * 2 + gl
                            yield ('l', lambda stg=stg, gxd=gxd, t0c=t0c, gi4=gi4, gl=gl: \
                                nc.sync.dma_start(
                                    gxd[:, :].rearrange("p (t g b) -> p t g b",
                                                        g=4, b=32)[
                                        :, t0c:t0c + 16, gi4, :],
                                    stg[gl * 64:(gl + 1) * 64, :].rearrange(
                                        "p (t b) -> p t b", b=32)))

            def bulk_gx(li, src_dram, src_k, rhs_dty):
                bulk_load_weights(li)
                for ch in range(NCH):
                    for kind, op in bulk_chunk_ops(li, src_dram, src_k, ch):
                        op()

            # ---------- LSTM recurrence (skewed dual-chain pipeline) ----------
            def lstm_layer(li, hbuf_out, sched=None, lb=3):
                """sched: dict block_idx -> list of micro-op generators; ops
                drain in order, <=1 heavy + <=lb light per step, at iter end."""
                from collections import deque
                pending = deque()

                def drain(hb, lb):
                    while pending:
                        kind, fn = pending[0]
                        if kind == 'h':
                            if hb <= 0:
                                break
                            hb -= 1
                        else:
                            if lb <= 0:
                                break
                            lb -= 1
                        pending.popleft()
                        fn()

                gxf, gxb = dt[f'gx{li}_f'], dt[f'gx{li}_b']
                NB = T // 8
                cts = {}
                hprev = {}
                for g2 in range(2):
                    cts[g2] = hcpool.tile([128, 16], F32, tag=f"c{g2}", name=f"c{g2}")
                    nc.vector.memset(cts[g2][:], 0.0)
                    hprev[g2] = zeros16
                gxt = {}
                hring = {}
                pss = {}
                sig = {}
                ths = {}

                def load_block(blk):
                    t0 = blk * 8
                    g = gxpool.tile([128, 8 * 128], BF16, tag="gx")
                    nc.sync.dma_start(g[0:64, :], gxf[:, t0 * 128:(t0 + 8) * 128])
                    # bwd: reversed-t read so slot k holds t = T-1-t0-k
                    nc.sync.dma_start(
                        g[64:128, :].rearrange("p (s f) -> p s f", f=128),
                        gxb[:, :].rearrange("p (t f) -> p t f", f=128)[
                            :, T - 1 - t0:T - 9 - t0 if T - 9 - t0 >= 0 else None:-1, :])
                    gxt[blk] = g
                    hring[blk] = {
                        g2: hcpool.tile([128, 8 * 16], F32, tag=f"hr{g2}",
                                        name=f"hr{g2}") for g2 in range(2)}

                def dump_block(blk):
                    t0 = blk * 8
                    for g2 in range(2):
                        bs = g2 * 16
                        hr = hring[blk][g2]
                        nc.sync.dma_start(
                            hbuf_out[0:64, :].rearrange("p (t b) -> p t b", b=BL)[
                                :, t0:t0 + 8, bs:bs + 16],
                            hr[0:64, :].rearrange("p (s b) -> p s b", b=16))
                        nc.sync.dma_start(
                            hbuf_out[64:128, :].rearrange("p (t b) -> p t b", b=BL)[
                                :, T - 1 - t0:T - 9 - t0 if T - 9 - t0 >= 0 else None:-1,
                                bs:bs + 16],
                            hr[64:128, :].rearrange("p (s b) -> p s b", b=16))
                    del gxt[blk], hring[blk]

                def S1(g2, k):     # PE: inject gx + accumulate whh gates
                    blk, kk = divmod(k, 8)
                    bs = g2 * 16
                    ps = pspool.tile([128, 64], F32, tag=f"lps{g2}",
                                     name=f"lps{g2}")
                    gxt_v = gxt[blk][:].rearrange("p (s g b) -> p s g b",
                                                  g=4, b=32)
                    nc.tensor.matmul(
                        ps[:].rearrange("p (g b) -> p g b", g=4),
                        ident128[:], gxt_v[:, kk, :, bs:bs + 16],
                        start=True, stop=False)
                    for gi in range(4):
                        nc.tensor.matmul(
                            ps[:, gi * 16:(gi + 1) * 16],
                            whh[(li, gi)][:], hprev[g2][:],
                            start=False, stop=(gi == 3), skip_group_check=True)
                    pss[g2] = ps

                def S2(g2, k):     # Act: all-gate sigmoid
                    s = sigpool.tile([128, 64], F32, tag=f"sig{g2}",
                                     name=f"sig{g2}")
                    nc.scalar.activation(s[:], pss[g2][:], AF.Sigmoid)
                    sig[g2] = s

                def S3(g2, k):     # DVE+Pool: cell-state update
                    s = sig[g2]
                    A = sigpool.tile([128, 16], F32, tag=f"A{g2}", name=f"A{g2}")
                    nc.vector.tensor_tensor(A[:], s[:, 0:16], s[:, 32:48],
                                            OP.mult)
                    Bt = sigpool.tile([128, 16], F32, tag=f"B{g2}", name=f"B{g2}")
                    nc.vector.scalar_tensor_tensor(Bt[:], A[:], 2.0, s[:, 0:16],
                                                   OP.mult, OP.subtract)
                    Ct = sigpool.tile([128, 16], F32, tag=f"C{g2}", name=f"C{g2}")
                    nc.gpsimd.tensor_tensor(Ct[:], s[:, 16:32], cts[g2][:],
                                            OP.mult)
                    cn = hcpool.tile([128, 16], F32, tag=f"c{g2}", name=f"c{g2}")
                    nc.vector.tensor_tensor(cn[:], Bt[:], Ct[:], OP.add)
                    cts[g2] = cn

                def S4(g2, k):     # Act: tanh(c)
                    th = sigpool.tile([128, 16], F32, tag=f"th{g2}",
                                      name=f"th{g2}")
                    nc.scalar.activation(th[:], cts[g2][:], AF.Tanh)
                    ths[g2] = th

                def S5(g2, k):     # DVE: h = o * tanh(c) into ring slot
                    blk, kk = divmod(k, 8)
                    hn = hring[blk][g2][:, kk * 16:(kk + 1) * 16]
                    nc.vector.tensor_tensor(hn, sig[g2][:, 48:64], ths[g2][:],
                                            OP.mult)
                    hprev[g2] = hn

                for k in range(T):
                    blk, kk = divmod(k, 8)
                    if kk == 0:
                        if sched:
                            for gen in sched.get(blk, []):
                                pending.extend(gen)
                        load_block(blk)
                    S1(0, k)
                    if k > 0:
                        S4(1, k - 1)
                        S5(1, k - 1)
                        if kk == 0:
                            dump_block(blk - 1)
                    S2(0, k)
                    S1(1, k)
                    S3(0, k)
                    S2(1, k)
                    S4(0, k)
                    S3(1, k)
                    S5(0, k)
                    drain(1, lb)
                S4(1, T - 1)
                S5(1, T - 1)
                dump_block(NB - 1)
                drain(10 ** 9, 10 ** 9)

            # ---------- emissions (micro-op generator) ----------
            def emissions_chunk_ops(hsrc, ch):
                rhs = rhspool.tile([128, 512], F32R, tag="erhs")
                yield ('l', lambda: nc.sync.dma_start(
                    rhs[:], hsrc[:, ch * 512:(ch + 1) * 512]))
                psb = pspool.tile([128, 512], F32, tag="big", name="epsb")
                ps = psb[0:CP, :]
                yield ('h', lambda: nc.tensor.matmul(ps, linWT[:], rhs[:],
                                                     start=True, stop=True))
                eo = epool.tile([CP, 512], F32, tag="eo")
                yield ('l', lambda: nc.scalar.activation(eo[:], ps, AF.Identity,
                                                         bias=linB[:]))
                for k in range(4):
                    psTb = pspool.tile([128, 64], F32, tag="epsT", name="psTb", bufs=1)
                    psT = psTb[:, 0:CP]
                    yield ('l', lambda psT=psT, k=k: nc.tensor.transpose(
                        psT, eo[:, k * 128:(k + 1) * 128], ident44[:]))
                    estg = epool.tile([128, CP], F32, tag="estg")
                    if k % 2 == 0:
                        yield ('l', lambda psT=psT, estg=estg: \
                               nc.scalar.activation(estg[:], psT, AF.Copy))
                    else:
                        yield ('l', lambda psT=psT, estg=estg: \
                               nc.vector.tensor_copy(estg[:], psT))
                    tb = ch * 4 + k
                    ed = dt['e_d'][:, :, :].rearrange("t p c -> (t p) c")
                    yield ('l', lambda estg=estg, tb=tb, ed=ed: \
                           nc.sync.dma_start(
                        ed[:, :].rearrange("(t b q) c -> (t b) q c", b=32, q=4)[
                            tb * 128:(tb + 1) * 128, :, :],
                        estg[:].unsqueeze(1).broadcast_to([128, 4, CP])))

            def emissions(hsrc):
                for ch in range(NCH):
                    for kind, op in emissions_chunk_ops(hsrc, ch):
                        op()
            # ---------- viterbi forward ----------            # ---------- viterbi forward ----------
            def viterbi_fwd():
                ed = dt['e_d'][:, :, :]
                sd = dt['score_d'][:, :, :]
                transB_v = transB[:].rearrange("p (w c) -> p w c", c=CP)
                scoreR = {}
                et = {}
                pend_copy = None
                prev_ps = None
                nblk = T // 4
                for bk in range(nblk):
                    t0 = bk * 4
                    et[bk] = vitpool.tile([128, 4 * CP], F32, tag="ein", name="ein")
                    nc.sync.dma_start(
                        et[bk][:].rearrange("p (s c) -> p s c", c=CP),
                        ed[t0:t0 + 4].rearrange("t p c -> p t c"))
                    scoreR[bk] = vitpool.tile([128, 4 * CP], F32, tag="sring",
                                              name="sring")
                    for k in range(4):
                        t = t0 + k
                        e_sl = et[bk][:, k * CP:(k + 1) * CP]
                        out_sl = scoreR[bk][:, k * CP:(k + 1) * CP]
                        if t == 0:
                            nc.vector.tensor_tensor(out_sl, startRep[:], e_sl, OP.add)
                            continue
                        prev = scoreR[bk][:, 0:CP] if t == 1 else \
                            prev_ps[:, 0:CP]
                        cand = vitpool.tile([128, CW * C], F32, tag="cand")
                        cand_v = cand[:].rearrange("p (w c) -> p w c", c=C)
                        nc.vector.tensor_tensor(
                            cand_v,
                            prev[:, 0:C].unsqueeze(1).broadcast_to([128, CW, C]),
                            transB_v[:, :, 0:C], OP.add)
                        bsh = vitpool.tile([128, CW], F32, tag="bsh")
                        nc.vector.tensor_reduce(
                            bsh[:], cand_v, op=OP.max, axis=AX.X)
                        # score(t) = best + e built fully in PSUM: e seeded by
                        # an identity mm (start=True), then 4 gather mms
                        # accumulate the distributed best (single group).
                        # The chain reads score straight from PSUM.
                        psc = pspool.tile([128, 64], F32, tag="lps0",
                                          name="vfps")
                        nc.tensor.matmul(psc[:, 0:CP], ident128[:], e_sl,
                                         start=True, stop=False,
                                         skip_group_check=True)
                        for g in range(4):
                            nc.tensor.matmul(
                                psc[:, g * CW:(g + 1) * CW],
                                gatherG[:, g * 128:(g + 1) * 128], bsh[:],
                                start=False, stop=(g == 3),
                                skip_group_check=True)
                        # history copy (Act, off-chain), deferred one step so
                        # tile-granular tracking can't stall the next cand
                        if pend_copy is not None:
                            nc.scalar.activation(pend_copy[0], pend_copy[1],
                                                 AF.Copy)
                            pend_copy = None
                        if k == 0 and bk >= 1:
                            nc.sync.dma_start(
                                sd[t0 - 4:t0].rearrange("t p c -> p t c"),
                                scoreR[bk - 1][:].rearrange(
                                    "p (s c) -> p s c", c=CP))
                        pend_copy = (out_sl, psc[:, 0:CP])
                        prev_ps = psc
                    if bk == nblk - 1:
                        if pend_copy is not None:
                            nc.scalar.activation(pend_copy[0], pend_copy[1],
                                                 AF.Copy)
                            pend_copy = None
                        nc.sync.dma_start(
                            sd[t0:t0 + 4].rearrange("t p c -> p t c"),
                            scoreR[bk][:].rearrange("p (s c) -> p s c", c=CP))
                    if bk >= 2:
                        del scoreR[bk - 2], et[bk - 2]
            # ---------- backtrace ----------            # ---------- backtrace ----------            # ---------- backtrace ----------
            def backtrace():
                # Single 32-wide chain: per step 1 PE mm (pu = vT @ transT) +
                # add + max-reduce + is_ge + PE transpose + Act copy (vT).
                sd = dt['score_d'][:, :, :]
                ohd = dt['oh_d'][:, :, :]
                BK = 8
                vT = None
                for bk in range(T // BK - 1, -1, -1):
                    t0 = bk * BK   # block covers t0 .. t0+7
                    sc = vitpool.tile([32, BK * CP], F32, tag="sc", name="sc")
                    nc.sync.dma_start(
                        sc[:].rearrange("p (s c) -> p s c", c=CP),
                        sd[t0:t0 + BK].rearrange(
                            "t (b q) c -> b t q c", q=4)[:, :, 0, :])
                    ohr = vitpool.tile([32, BK * CP], F32, tag="ohr", name="ohr")
                    ur = vitpool.tile([32, BK * CP], F32, tag="ur", name="ur")
                    for k in range(BK - 1, -1, -1):
                        t = t0 + k
                        sc_sl = sc[:, k * CP:(k + 1) * CP]
                        u_sl = ur[:, k * CP:(k + 1) * CP]
                        m2 = vitpool.tile([32, 1], F32, tag="m2", name="m2")
                        if t == T - 1:
                            nc.vector.tensor_tensor(u_sl, sc_sl, endRep[:],
                                                    OP.add)
                        else:
                            pub = pspool.tile([64, 64], F32, tag="bt",
                                              name="pub", bufs=1)
                            pu = pub[0:32, 0:CP]
                            nc.tensor.matmul(pu, vT[:], transT[:],
                                             start=True, stop=True)
                            nc.vector.tensor_tensor(u_sl, sc_sl, pu, OP.add)
                        nc.vector.tensor_reduce(m2[:], u_sl, op=OP.max,
                                                axis=AX.X)
                        oh_sl = ohr[:, k * CP:(k + 1) * CP]
                        nc.vector.tensor_tensor(
                            oh_sl, u_sl, m2[:].broadcast_to([32, CP]),
                            OP.is_ge)
                        if t > 0:
                            pT = pspool.tile([64, 64], F32, tag="bt",
                                             name="pT", bufs=1)
                            nc.tensor.transpose(pT[0:CP, 0:32], oh_sl, ident32[:])
                            vT = sigpool.tile([C, 32], F32, tag="vT",
                                              name="vT")
                            nc.vector.tensor_copy(vT[:], pT[0:C, 0:32])
                    nc.sync.dma_start(
                        ohd[t0:t0 + BK, :, :].rearrange("t p c -> p t c"),
                        ohr[:].rearrange("p (s c) -> p s c", c=CP))
            # ---------- extract tags ----------            # ---------- extract tags ----------            # ---------- extract tags ----------
            def extract():
                ohd = dt['oh_d'][:, :, :].rearrange("t b c -> (t b) c")
                NJT = T // 4
                JC = min(16, NJT)
                for jc in range(NJT // JC):
                    ohch = vitpool.tile([128, JC * CP], F32, tag="ohch", bufs=2)
                    nc.sync.dma_start(
                        ohch[:].rearrange("p (j c) -> p j c", c=CP),
                        ohd[:, :].rearrange("(j p) c -> p j c", p=128)[
                            :, jc * JC:(jc + 1) * JC, :])
                    prod = vitpool.tile([128, JC * CP], F32, tag="prod", bufs=2)
                    nc.vector.tensor_tensor(
                        prod[:].rearrange("p (j c) -> p j c", c=CP),
                        ohch[:].rearrange("p (j c) -> p j c", c=CP),
                        iotaRep[:].unsqueeze(1).broadcast_to([128, JC, CP]),
                        OP.mult)
                    tf = vitpool.tile([128, JC], F32, tag="tf")
                    nc.vector.tensor_reduce(
                        tf[:], prod[:].rearrange("p (j c) -> p j c", c=CP),
                        op=OP.add, axis=AX.X)
                    ti = vitpool.tile([128, JC], I32, tag="ti")
                    nc.vector.tensor_copy(ti[:], tf[:])
                    for tl in range(4):
                        nc.sync.dma_start(
                            dt['tags'][:, :].rearrange("b (j f) -> b j f", f=4)[
                                :, jc * JC:(jc + 1) * JC, tl],
                            ti[tl * 32:(tl + 1) * 32, :])

            def next_layer_sched(gen_for_chunk):
                """Feed one chunk per block from block 66: chunk 31-j at
                block 66+2j, chunk 32+j at 67+2j (respects hbuf dump order:
                chunk c readable once dumps of 8t-blocks <= max(2c+1,127-2c)
                are emitted, i.e. from block max(2c+2, 128-2c))."""
                s = {}
                for j in range(31):
                    s.setdefault(66 + 2 * j, []).append(gen_for_chunk(31 - j))
                    s.setdefault(67 + 2 * j, []).append(gen_for_chunk(32 + j))
                # chunks 0 and 63 drain after the loop (post-drain)
                s.setdefault(10 ** 6, [])
                return s, [gen_for_chunk(0), gen_for_chunk(63)]

            ph = 63
            if ph == 63:
                bulk_load_weights(0)
                bulk_load_weights(1)
                bulk_load_weights(2)
                # bulk0: pairs (j, 63-j); j=0,1 fully up front, rest paced
                s0 = {}
                for j in range(2):
                    for ch in (j, 63 - j):
                        for kind, op in bulk_chunk_ops(0, dt['xT'], 40, ch):
                            op()
                for j in range(2, 32):
                    s0.setdefault(2 * (j - 2), []).extend([
                        bulk_chunk_ops(0, dt['xT'], 40, j),
                        bulk_chunk_ops(0, dt['xT'], 40, 63 - j)])
                # bulk1 into L0's tail
                sb1, tail1 = next_layer_sched(
                    lambda c: bulk_chunk_ops(1, dt['hbuf0'], 128, c))
                for blk, gens in sb1.items():
                    s0.setdefault(blk, []).extend(gens)
                lstm_layer(0, dt['hbuf0'], s0, lb=4)
                for g in tail1:
                    for kind, op in g:
                        op()
                # bulk2 into L1
                sb2, tail2 = next_layer_sched(
                    lambda c: bulk_chunk_ops(2, dt['hbuf1'], 128, c))
                lstm_layer(1, dt['hbuf1'], sb2, lb=4)
                for g in tail2:
                    for kind, op in g:
                        op()
                # emissions into L2 (2 chunks/block from block 66)
                se = {}
                for j in range(31):
                    se.setdefault(66 + 2 * j, []).append(
                        emissions_chunk_ops(dt['hbuf2'], 31 - j))
                    se.setdefault(67 + 2 * j, []).append(
                        emissions_chunk_ops(dt['hbuf2'], 32 + j))
                tail_e = [emissions_chunk_ops(dt['hbuf2'], 0),
                          emissions_chunk_ops(dt['hbuf2'], 63)]
                lstm_layer(2, dt['hbuf2'], se, lb=7)
                for g in tail_e:
                    for kind, op in g:
                        op()
                viterbi_fwd()
                backtrace()
                extract()
            else:
                if ph & 1:
                    bulk_gx(0, dt['xT'], 40, BF16)
                if ph & 2:
                    lstm_layer(0, dt['hbuf0'])
                    if ph & 1:
                        bulk_gx(1, dt['hbuf0'], 128, BF16)
                    lstm_layer(1, dt['hbuf1'])
                    if ph & 1:
                        bulk_gx(2, dt['hbuf1'], 128, BF16)
                    lstm_layer(2, dt['hbuf2'])
                if ph & 4:
                    emissions(dt['hbuf2'])
                if ph & 8:
                    viterbi_fwd()
                if ph & 16:
                    backtrace()
                if ph & 32:
                    extract()

    legalize_waits(nc)
    return nc


def make_in_map(inputs, cid, T, wd):
    m = {'xT': shard_x(inputs['x'], cid, T)}
    m.update(wd)
    return m


_CACHE = {}


def kernel(x, w_ih_l0, w_hh_l0, b_l0, w_ih_r, w_hh_r, b_r,
           lin_w, lin_b, crf_start, crf_end, crf_trans):
    """Full BiLSTM-CRF on 8 NeuronCores, data-parallel over the batch."""
    from concourse.bass_utils import run_bass_kernel_spmd
    T = 1024
    if 'nc' not in _CACHE:
        _CACHE['nc'] = build_nc(T)
    nc = _CACHE['nc']
    wd = prep_weights(w_ih_l0, w_hh_l0, b_l0, w_ih_r, w_hh_r, b_r,
                      lin_w, lin_b, crf_start, crf_end, crf_trans)
    in_maps = []
    for cid in range(NCORES):
        m = {'xT': shard_x(x, cid, T)}
        m.update(wd)
        in_maps.append(m)
    res = run_bass_kernel_spmd(nc, in_maps, core_ids=list(range(NCORES)))
    tags = np.concatenate([res.results[c]['tags'] for c in range(NCORES)], axis=0)
    return np.ascontiguousarray(tags.astype(np.int32))



# revision 58
# speedup vs baseline: 1.0039x; 1.0039x over previous
"""Development version of the full-device BiLSTM-CRF kernel. See design notes.

Layouts (per core, BL=32 sequences):
 - LSTM gate-major: partitions = [fwd feat 64; bwd feat 64]; psum free =
   (pair-parity, gate, batch32). Two 16-seq groups pipeline the step chain.
 - gx bulk-matmul'd (f32r/bf16, N=512) into DRAM per direction; identity
   matmul accumulates into PSUM per step pair.
 - Viterbi forward: cp sharded 4-way across partition groups; score/e/onehot
   histories time-folded [128, T/4 * 41] (partition group = t%4).
 - Backtrace: onehot chain via PE matmul with trans^T, TTR fused add+max.
"""
import sys
sys.path.insert(0, '/opt/trn_rl_repo')
import numpy as np
import ml_dtypes
import concourse.bass as bass
import concourse.mybir as mybir
from concourse.tile import TileContext

F32 = mybir.dt.float32
F32R = mybir.dt.float32  # f32r reverted: interp models f32r with reduced precision
BF16 = mybir.dt.float32  # precision experiment: all-f32
I32 = mybir.dt.int32
AF = mybir.ActivationFunctionType
OP = mybir.AluOpType
AX = mybir.AxisListType

B, D_IN, HID, C = 256, 39, 128, 41
H = HID // 2
G4 = 4 * H
NCORES = 8
BL = B // NCORES
CP = 44
NG = 4
CW = 11
NEG = -1.0e30


def legalize_waits(nc):
    n = 0
    for _, bbw in nc.bb_map.items():
        il = bbw.bb.instructions
        out = []
        for i in il:
            si = getattr(i, 'sync_info', None)
            ow = list(si.on_wait) if (si is not None and si.on_wait) else []
            if len(ow) > 1:
                for w in ow[:-1]:
                    n += 1
                    es = mybir.InstEventSemaphore(
                        name=f"legwait-{n}-{i.name}", engine=i.engine, ins=[], outs=[],
                        sync_info=mybir.SyncInfo(on_wait=[w], on_update=[]))
                    out.append(es)
                i.sync_info = mybir.SyncInfo(on_wait=[ow[-1]], on_update=list(si.on_update or []))
            out.append(i)
        bbw.bb.instructions = out
    return n


def prep_weights(w_ih_l0, w_hh_l0, b_l0, w_ih_r, w_hh_r, b_r,
                 lin_w, lin_b, crf_start, crf_end, crf_trans):
    """Gate order i,f,g,o. g rows scaled x2 (tanh(z) = 2*sigmoid(2z)-1)."""
    d = {}

    def gscale(m):
        m = np.asarray(m, np.float32).copy()
        m[2 * H:3 * H] *= 2.0
        return m

    for di, nm in ((0, 'f'), (1, 'b')):
        w = gscale(w_ih_l0[di])
        bb = gscale(b_l0[di])
        d[f'wx0_{nm}'] = np.concatenate([w.T, bb[None, :]], 0).astype(np.float32)
    for li in (0, 1):
        for di, nm in ((0, 'f'), (1, 'b')):
            w = gscale(w_ih_r[li, di])
            bb = gscale(b_r[li, di])
            d[f'wx{li+1}_{nm}'] = np.ascontiguousarray(w.T).astype(np.float32)
            d[f'bias{li+1}_{nm}'] = bb[None, :].astype(np.float32)
    for li in range(3):
        whh = np.asarray(w_hh_l0) if li == 0 else np.asarray(w_hh_r[li - 1])
        for gi in range(4):
            blk = np.zeros((128, 128), np.float32)
            sc = 2.0 if gi == 2 else 1.0
            blk[0:64, 0:64] = sc * whh[0, gi * H:(gi + 1) * H, :].T
            blk[64:128, 64:128] = sc * whh[1, gi * H:(gi + 1) * H, :].T
            d[f'whh{li}_{gi}'] = blk.astype(np.float32)
    d['ident128'] = np.eye(128, dtype=np.float32)
    d['ident16'] = np.eye(16, dtype=np.float32)
    d['ident32'] = np.eye(32, dtype=np.float32)
    d['ident44'] = np.eye(CP, dtype=np.float32)
    lw = np.zeros((HID, CP), np.float32)
    lw[:, :C] = np.asarray(lin_w, np.float32).T
    d['linWT'] = lw.astype(np.float32)
    lb = np.full((CP, 1), NEG, np.float32)
    lb[:C, 0] = np.asarray(lin_b, np.float32)
    d['linB'] = lb
    tr = np.asarray(crf_trans, np.float32)
    # transB_cn[p=(b,g), (ci, cp)] = trans[cp, g*CW+ci], NEG for pads
    transB = np.full((128, CW, CP), NEG, np.float32)
    for g in range(NG):
        for ci in range(CW):
            cn = g * CW + ci
            if cn < C:
                for b in range(32):
                    transB[b * 4 + g, ci, :C] = tr[:, cn]
    d['transB'] = transB.reshape(128, CW * CP)
    trT = np.full((C, CP), NEG, np.float32)
    trT[:, :C] = tr.T  # [cn, cp]
    d['transT'] = trT
    d['transThi'] = np.ascontiguousarray(trT[32:41])  # cn 32..40 rows
    st = np.full((128, CP), NEG, np.float32)
    st[:, :C] = np.asarray(crf_start, np.float32)[None, :]
    d['startRep'] = st
    en = np.full((32, CP), NEG, np.float32)
    en[:, :C] = np.asarray(crf_end, np.float32)[None, :]
    d['endRep'] = en
    io = np.zeros((128, CP), np.float32)
    io[:, :C] = np.arange(C, dtype=np.float32)[None, :]
    d['iotaRep'] = io
    d['onesrow'] = np.ones((1, 512), np.float32)
    gG = np.zeros((128, 4, 128), np.float32)
    for b in range(32):
        for g in range(4):
            for gp in range(4):
                gG[b * 4 + g, g, b * 4 + gp] = 1.0
    d['gatherG'] = gG.reshape(128, 512)
    d['zeros16'] = np.zeros((128, 16), np.float32)
    return d


def shard_x(x, cid, T):
    xs = np.asarray(x, np.float32)[cid * BL:(cid + 1) * BL, :T]
    xt = np.empty((D_IN + 1, T * BL), np.float32)
    xt[D_IN] = 1.0
    xt[:D_IN] = xs.transpose(2, 1, 0).reshape(D_IN, T * BL)
    return xt.astype(np.float32)


def build_nc(T):
    R = BL * T
    TJ = T // 4
    NCH = R // 512
    nc = bass.Bass()
    dt = {}

    def din(name, shape, dty=F32):
        dt[name] = nc.dram_tensor(name, shape, dty, kind="ExternalInput")

    din('xT', [D_IN + 1, R], F32R)
    din('wx0_f', [40, 256], F32R); din('wx0_b', [40, 256], F32R)
    for li in (1, 2):
        for nm in ('f', 'b'):
            din(f'wx{li}_{nm}', [128, 256], F32R)
            din(f'bias{li}_{nm}', [1, 256], F32R)
    for li in range(3):
        for gi in range(4):
            din(f'whh{li}_{gi}', [128, 128], F32)
    din('ident128', [128, 128], BF16); din('ident16', [16, 16]); din('ident32', [32, 32]); din('ident44', [CP, CP])
    din('linWT', [HID, CP], F32R); din('linB', [CP, 1])
    din('transB', [128, CW * CP]); din('transT', [C, CP])
    din('transThi', [9, CP])
    din('startRep', [128, CP]); din('endRep', [32, CP]); din('iotaRep', [128, CP])
    din('onesrow', [1, 512], F32R); din('zeros16', [128, 16], F32)
    din('gatherG', [128, 512])

    def scratch(name, shape, dty=F32):
        dt[name] = nc.dram_tensor(name, shape, dty, kind="Internal")

    for li3 in range(3):
        scratch(f'gx{li3}_f', [64, T * 128], BF16)
        scratch(f'gx{li3}_b', [64, T * 128], BF16)
    scratch('hbuf0', [HID, R], F32R)
    scratch('hbuf1', [HID, R], F32R)
    scratch('hbuf2', [HID, R], F32R)
    scratch('e_d', [T, 128, CP])
    scratch('score_d', [T, 128, CP])
    scratch('oh_d', [T, 32, CP])
    dt['tags'] = nc.dram_tensor('tags', [BL, T], I32, kind="ExternalOutput")

    with TileContext(nc) as tc:
        with tc.tile_pool(name="const", bufs=1) as cpool, \
             tc.tile_pool(name="wpool", bufs=1) as wpool, \
             tc.tile_pool(name="hist", bufs=1) as hpool, \
             tc.tile_pool(name="bulk_rhs", bufs=4) as rhspool, \
             tc.tile_pool(name="gx", bufs=8) as gxpool, \
             tc.tile_pool(name="psum", bufs=2, space="PSUM") as pspool, \
             tc.tile_pool(name="sig", bufs=8) as sigpool, \
             tc.tile_pool(name="hc", bufs=8) as hcpool, \
             tc.tile_pool(name="vit", bufs=4) as vitpool, \
             tc.tile_pool(name="emis", bufs=2) as epool:

            def load_const(nm, shape, dty=F32):
                t = cpool.tile(shape, dty, tag=nm)
                nc.sync.dma_start(t[:], dt[nm][:])
                return t

            ident128 = load_const('ident128', [128, 128], BF16)
            ident16 = load_const('ident16', [16, 16])
            ident32 = load_const('ident32', [32, 32])
            ident44 = load_const('ident44', [CP, CP])
            linWT = load_const('linWT', [HID, CP], F32R)
            linB = load_const('linB', [CP, 1])
            transB = load_const('transB', [128, CW * CP])
            transT = load_const('transT', [C, CP])
            transThi = load_const('transThi', [9, CP])
            startRep = load_const('startRep', [128, CP])
            endRep = load_const('endRep', [32, CP])
            iotaRep = load_const('iotaRep', [128, CP])
            gatherG = load_const('gatherG', [128, 512])
            whh = {}
            for li in range(3):
                for gi in range(4):
                    whh[(li, gi)] = load_const(f'whh{li}_{gi}', [128, 128], F32)
            onesrow = load_const('onesrow', [1, 512], F32R)
            zeros16 = load_const('zeros16', [128, 16], F32)

            scoreRep = hpool.tile([128, CP], F32, tag="scoreRep")

            # ---------- bulk gx (micro-op generator for interleaving) ----------
            bulk_state = {}

            def bulk_load_weights(li):
                wx = {}
                bias = {}
                for nm in ('f', 'b'):
                    wx[nm] = wpool.tile([40 if li == 0 else 128, 256], F32R,
                                        tag=f"wx{li}_{nm}", name=f"wx{li}{nm}")
                    nc.sync.dma_start(wx[nm][:], dt[f'wx{li}_{nm}'][:])
                    if li > 0:
                        bias[nm] = wpool.tile([1, 256], F32R, tag=f"bias{li}_{nm}", name=f"bias{li}{nm}")
                        nc.sync.dma_start(bias[nm][:], dt[f'bias{li}_{nm}'][:])
                bulk_state[li] = (wx, bias)

            def bulk_chunk_ops(li, src_dram, src_k, ch):
                """Yield micro-closures; caller drains them spread over time."""
                wx, bias = bulk_state[li]
                rhs = rhspool.tile([src_k, 512], F32R, tag=f"rhs{li}")
                yield ('l', lambda: nc.sync.dma_start(
                    rhs[:], src_dram[:, ch * 512:(ch + 1) * 512]))
                for nm in ('f', 'b'):
                    for pr in range(2):
                        ps = pspool.tile([128, 512], F32, tag="big", name="bps")
                        # matmuls split into 256-col halves so each drained
                        # piece fits the PE idle window of one LSTM step
                        for hh in range(2):
                            sl = slice(hh * 256, (hh + 1) * 256)
                            if li == 0:
                                yield ('h', lambda ps=ps, nm=nm, pr=pr, sl=sl:
                                       nc.tensor.matmul(
                                    ps[:, sl], wx[nm][:, pr * 128:(pr + 1) * 128],
                                    rhs[:, sl], start=True, stop=True,
                                    skip_group_check=True))
                            else:
                                yield ('h', lambda ps=ps, nm=nm, pr=pr, sl=sl:
                                       nc.tensor.matmul(
                                    ps[:, sl], wx[nm][:, pr * 128:(pr + 1) * 128],
                                    rhs[:, sl], start=True, stop=False,
                                    skip_group_check=True))
                                yield ('h', lambda ps=ps, nm=nm, pr=pr, sl=sl:
                                       nc.tensor.matmul(
                                    ps[:, sl], bias[nm][:, pr * 128:(pr + 1) * 128],
                                    onesrow[:, sl], start=False, stop=True,
                                    skip_group_check=True))
                        stg = rhspool.tile([128, 512], BF16, tag="gxstg",
                                           name="gxstg")
                        if (ch + pr) % 2 == 0:
                            yield ('l', lambda ps=ps, stg=stg: nc.scalar.activation(
                                stg[:], ps[:], AF.Copy))
                        else:
                            yield ('l', lambda ps=ps, stg=stg: nc.vector.tensor_copy(
                                stg[:], ps[:]))
                        gxd = dt[f'gx{li}_{nm}']
                        t0c = ch * 16
                        for gl in range(2):
                            gi4 = pr * 2 + gl
                            yield ('l', lambda stg=stg, gxd=gxd, t0c=t0c, gi4=gi4, gl=gl: \
                                nc.sync.dma_start(
                                    gxd[:, :].rearrange("p (t g b) -> p t g b",
                                                        g=4, b=32)[
                                        :, t0c:t0c + 16, gi4, :],
                                    stg[gl * 64:(gl + 1) * 64, :].rearrange(
                                        "p (t b) -> p t b", b=32)))

            def bulk_gx(li, src_dram, src_k, rhs_dty):
                bulk_load_weights(li)
                for ch in range(NCH):
                    for kind, op in bulk_chunk_ops(li, src_dram, src_k, ch):
                        op()

            # ---------- LSTM recurrence (skewed dual-chain pipeline) ----------
            def lstm_layer(li, hbuf_out, sched=None, lb=3):
                """sched: dict block_idx -> list of micro-op generators; ops
                drain in order, <=1 heavy + <=lb light per step, at iter end."""
                from collections import deque
                pending = deque()

                def drain(hb, lb):
                    while pending:
                        kind, fn = pending[0]
                        if kind == 'h':
                            if hb <= 0:
                                break
                            hb -= 1
                        else:
                            if lb <= 0:
                                break
                            lb -= 1
                        pending.popleft()
                        fn()

                gxf, gxb = dt[f'gx{li}_f'], dt[f'gx{li}_b']
                NB = T // 8
                cts = {}
                hprev = {}
                for g2 in range(2):
                    cts[g2] = hcpool.tile([128, 16], F32, tag=f"c{g2}", name=f"c{g2}")
                    nc.vector.memset(cts[g2][:], 0.0)
                    hprev[g2] = zeros16
                gxt = {}
                hring = {}
                pss = {}
                sig = {}
                ths = {}

                def load_block(blk):
                    t0 = blk * 8
                    g = gxpool.tile([128, 8 * 128], BF16, tag="gx")
                    nc.sync.dma_start(g[0:64, :], gxf[:, t0 * 128:(t0 + 8) * 128])
                    # bwd: reversed-t read so slot k holds t = T-1-t0-k
                    nc.sync.dma_start(
                        g[64:128, :].rearrange("p (s f) -> p s f", f=128),
                        gxb[:, :].rearrange("p (t f) -> p t f", f=128)[
                            :, T - 1 - t0:T - 9 - t0 if T - 9 - t0 >= 0 else None:-1, :])
                    gxt[blk] = g
                    hring[blk] = {
                        g2: hcpool.tile([128, 8 * 16], F32, tag=f"hr{g2}",
                                        name=f"hr{g2}") for g2 in range(2)}

                def dump_block(blk):
                    t0 = blk * 8
                    for g2 in range(2):
                        bs = g2 * 16
                        hr = hring[blk][g2]
                        nc.sync.dma_start(
                            hbuf_out[0:64, :].rearrange("p (t b) -> p t b", b=BL)[
                                :, t0:t0 + 8, bs:bs + 16],
                            hr[0:64, :].rearrange("p (s b) -> p s b", b=16))
                        nc.sync.dma_start(
                            hbuf_out[64:128, :].rearrange("p (t b) -> p t b", b=BL)[
                                :, T - 1 - t0:T - 9 - t0 if T - 9 - t0 >= 0 else None:-1,
                                bs:bs + 16],
                            hr[64:128, :].rearrange("p (s b) -> p s b", b=16))
                    del gxt[blk], hring[blk]

                def S1(g2, k):     # PE: inject gx + accumulate whh gates
                    blk, kk = divmod(k, 8)
                    bs = g2 * 16
                    ps = pspool.tile([128, 64], F32, tag=f"lps{g2}",
                                     name=f"lps{g2}")
                    gxt_v = gxt[blk][:].rearrange("p (s g b) -> p s g b",
                                                  g=4, b=32)
                    nc.tensor.matmul(
                        ps[:].rearrange("p (g b) -> p g b", g=4),
                        ident128[:], gxt_v[:, kk, :, bs:bs + 16],
                        start=True, stop=False)
                    for gi in range(4):
                        nc.tensor.matmul(
                            ps[:, gi * 16:(gi + 1) * 16],
                            whh[(li, gi)][:], hprev[g2][:],
                            start=False, stop=(gi == 3), skip_group_check=True)
                    pss[g2] = ps

                def S2(g2, k):     # Act: all-gate sigmoid
                    s = sigpool.tile([128, 64], F32, tag=f"sig{g2}",
                                     name=f"sig{g2}")
                    nc.scalar.activation(s[:], pss[g2][:], AF.Sigmoid)
                    sig[g2] = s

                def S3(g2, k):     # DVE+Pool: cell-state update
                    s = sig[g2]
                    A = sigpool.tile([128, 16], F32, tag=f"A{g2}", name=f"A{g2}")
                    nc.vector.tensor_tensor(A[:], s[:, 0:16], s[:, 32:48],
                                            OP.mult)
                    Bt = sigpool.tile([128, 16], F32, tag=f"B{g2}", name=f"B{g2}")
                    nc.vector.scalar_tensor_tensor(Bt[:], A[:], 2.0, s[:, 0:16],
                                                   OP.mult, OP.subtract)
                    Ct = sigpool.tile([128, 16], F32, tag=f"C{g2}", name=f"C{g2}")
                    nc.gpsimd.tensor_tensor(Ct[:], s[:, 16:32], cts[g2][:],
                                            OP.mult)
                    cn = hcpool.tile([128, 16], F32, tag=f"c{g2}", name=f"c{g2}")
                    nc.vector.tensor_tensor(cn[:], Bt[:], Ct[:], OP.add)
                    cts[g2] = cn

                def S4(g2, k):     # Act: tanh(c)
                    th = sigpool.tile([128, 16], F32, tag=f"th{g2}",
                                      name=f"th{g2}")
                    nc.scalar.activation(th[:], cts[g2][:], AF.Tanh)
                    ths[g2] = th

                def S5(g2, k):     # DVE: h = o * tanh(c) into ring slot
                    blk, kk = divmod(k, 8)
                    hn = hring[blk][g2][:, kk * 16:(kk + 1) * 16]
                    nc.vector.tensor_tensor(hn, sig[g2][:, 48:64], ths[g2][:],
                                            OP.mult)
                    hprev[g2] = hn

                for k in range(T):
                    blk, kk = divmod(k, 8)
                    if kk == 0:
                        if sched:
                            for gen in sched.get(blk, []):
                                pending.extend(gen)
                        load_block(blk)
                    S1(0, k)
                    if k > 0:
                        S4(1, k - 1)
                        S5(1, k - 1)
                        if kk == 0:
                            dump_block(blk - 1)
                    S2(0, k)
                    S1(1, k)
                    S3(0, k)
                    S2(1, k)
                    S4(0, k)
                    S3(1, k)
                    S5(0, k)
                    drain(2, lb)
                S4(1, T - 1)
                S5(1, T - 1)
                dump_block(NB - 1)
                drain(10 ** 9, 10 ** 9)

            # ---------- emissions (micro-op generator) ----------
            def emissions_chunk_ops(hsrc, ch):
                rhs = rhspool.tile([128, 512], F32R, tag="erhs")
                yield ('l', lambda: nc.sync.dma_start(
                    rhs[:], hsrc[:, ch * 512:(ch + 1) * 512]))
                psb = pspool.tile([128, 512], F32, tag="big", name="epsb")
                ps = psb[0:CP, :]
                for hh in range(2):
                    sl = slice(hh * 256, (hh + 1) * 256)
                    yield ('h', lambda sl=sl: nc.tensor.matmul(
                        ps[:, sl], linWT[:], rhs[:, sl],
                        start=True, stop=True, skip_group_check=True))
                eo = epool.tile([CP, 512], F32, tag="eo")
                yield ('l', lambda: nc.scalar.activation(eo[:], ps, AF.Identity,
                                                         bias=linB[:]))
                for k in range(4):
                    psTb = pspool.tile([128, 64], F32, tag="epsT", name="psTb", bufs=1)
                    psT = psTb[:, 0:CP]
                    yield ('l', lambda psT=psT, k=k: nc.tensor.transpose(
                        psT, eo[:, k * 128:(k + 1) * 128], ident44[:]))
                    estg = epool.tile([128, CP], F32, tag="estg")
                    if k % 2 == 0:
                        yield ('l', lambda psT=psT, estg=estg: \
                               nc.scalar.activation(estg[:], psT, AF.Copy))
                    else:
                        yield ('l', lambda psT=psT, estg=estg: \
                               nc.vector.tensor_copy(estg[:], psT))
                    tb = ch * 4 + k
                    ed = dt['e_d'][:, :, :].rearrange("t p c -> (t p) c")
                    yield ('l', lambda estg=estg, tb=tb, ed=ed: \
                           nc.sync.dma_start(
                        ed[:, :].rearrange("(t b q) c -> (t b) q c", b=32, q=4)[
                            tb * 128:(tb + 1) * 128, :, :],
                        estg[:].unsqueeze(1).broadcast_to([128, 4, CP])))

            def emissions(hsrc):
                for ch in range(NCH):
                    for kind, op in emissions_chunk_ops(hsrc, ch):
                        op()
            # ---------- viterbi forward ----------            # ---------- viterbi forward ----------
            def viterbi_fwd():
                ed = dt['e_d'][:, :, :]
                sd = dt['score_d'][:, :, :]
                transB_v = transB[:].rearrange("p (w c) -> p w c", c=CP)
                scoreR = {}
                et = {}
                pend_copy = None
                prev_ps = None
                nblk = T // 4
                for bk in range(nblk):
                    t0 = bk * 4
                    et[bk] = vitpool.tile([128, 4 * CP], F32, tag="ein", name="ein")
                    nc.sync.dma_start(
                        et[bk][:].rearrange("p (s c) -> p s c", c=CP),
                        ed[t0:t0 + 4].rearrange("t p c -> p t c"))
                    scoreR[bk] = vitpool.tile([128, 4 * CP], F32, tag="sring",
                                              name="sring")
                    for k in range(4):
                        t = t0 + k
                        e_sl = et[bk][:, k * CP:(k + 1) * CP]
                        out_sl = scoreR[bk][:, k * CP:(k + 1) * CP]
                        if t == 0:
                            nc.vector.tensor_tensor(out_sl, startRep[:], e_sl, OP.add)
                            continue
                        prev = scoreR[bk][:, 0:CP] if t == 1 else \
                            prev_ps[:, 0:CP]
                        cand = vitpool.tile([128, CW * C], F32, tag="cand")
                        cand_v = cand[:].rearrange("p (w c) -> p w c", c=C)
                        nc.vector.tensor_tensor(
                            cand_v,
                            prev[:, 0:C].unsqueeze(1).broadcast_to([128, CW, C]),
                            transB_v[:, :, 0:C], OP.add)
                        bsh = vitpool.tile([128, CW], F32, tag="bsh")
                        nc.vector.tensor_reduce(
                            bsh[:], cand_v, op=OP.max, axis=AX.X)
                        # score(t) = best + e built fully in PSUM: e seeded by
                        # an identity mm (start=True), then 4 gather mms
                        # accumulate the distributed best (single group).
                        # The chain reads score straight from PSUM.
                        psc = pspool.tile([128, 64], F32, tag="lps0",
                                          name="vfps")
                        nc.tensor.matmul(psc[:, 0:CP], ident128[:], e_sl,
                                         start=True, stop=False,
                                         skip_group_check=True)
                        for g in range(4):
                            nc.tensor.matmul(
                                psc[:, g * CW:(g + 1) * CW],
                                gatherG[:, g * 128:(g + 1) * 128], bsh[:],
                                start=False, stop=(g == 3),
                                skip_group_check=True)
                        # history copy (Act, off-chain), deferred one step so
                        # tile-granular tracking can't stall the next cand
                        if pend_copy is not None:
                            nc.scalar.activation(pend_copy[0], pend_copy[1],
                                                 AF.Copy)
                            pend_copy = None
                        if k == 0 and bk >= 1:
                            nc.sync.dma_start(
                                sd[t0 - 4:t0].rearrange("t p c -> p t c"),
                                scoreR[bk - 1][:].rearrange(
                                    "p (s c) -> p s c", c=CP))
                        pend_copy = (out_sl, psc[:, 0:CP])
                        prev_ps = psc
                    if bk == nblk - 1:
                        if pend_copy is not None:
                            nc.scalar.activation(pend_copy[0], pend_copy[1],
                                                 AF.Copy)
                            pend_copy = None
                        nc.sync.dma_start(
                            sd[t0:t0 + 4].rearrange("t p c -> p t c"),
                            scoreR[bk][:].rearrange("p (s c) -> p s c", c=CP))
                    if bk >= 2:
                        del scoreR[bk - 2], et[bk - 2]
            # ---------- backtrace ----------            # ---------- backtrace ----------            # ---------- backtrace ----------
            def backtrace():
                # Single 32-wide chain: per step 1 PE mm (pu = vT @ transT) +
                # add + max-reduce + is_ge + PE transpose + Act copy (vT).
                sd = dt['score_d'][:, :, :]
                ohd = dt['oh_d'][:, :, :]
                BK = 8
                vT = None
                for bk in range(T // BK - 1, -1, -1):
                    t0 = bk * BK   # block covers t0 .. t0+7
                    sc = vitpool.tile([32, BK * CP], F32, tag="sc", name="sc")
                    nc.sync.dma_start(
                        sc[:].rearrange("p (s c) -> p s c", c=CP),
                        sd[t0:t0 + BK].rearrange(
                            "t (b q) c -> b t q c", q=4)[:, :, 0, :])
                    ohr = vitpool.tile([32, BK * CP], F32, tag="ohr", name="ohr")
                    ur = vitpool.tile([32, BK * CP], F32, tag="ur", name="ur")
                    for k in range(BK - 1, -1, -1):
                        t = t0 + k
                        sc_sl = sc[:, k * CP:(k + 1) * CP]
                        u_sl = ur[:, k * CP:(k + 1) * CP]
                        m2 = vitpool.tile([32, 1], F32, tag="m2", name="m2")
                        if t == T - 1:
                            nc.vector.tensor_tensor(u_sl, sc_sl, endRep[:],
                                                    OP.add)
                        else:
                            pub = pspool.tile([64, 64], F32, tag="bt",
                                              name="pub", bufs=1)
                            pu = pub[0:32, 0:CP]
                            nc.tensor.matmul(pu, vT[:], transT[:],
                                             start=True, stop=True)
                            nc.vector.tensor_tensor(u_sl, sc_sl, pu, OP.add)
                        nc.vector.tensor_reduce(m2[:], u_sl, op=OP.max,
                                                axis=AX.X)
                        oh_sl = ohr[:, k * CP:(k + 1) * CP]
                        nc.vector.tensor_tensor(
                            oh_sl, u_sl, m2[:].broadcast_to([32, CP]),
                            OP.is_ge)
                        if t > 0:
                            pT = pspool.tile([64, 64], F32, tag="bt",
                                             name="pT", bufs=1)
                            nc.tensor.transpose(pT[0:CP, 0:32], oh_sl, ident32[:])
                            vT = sigpool.tile([C, 32], F32, tag="vT",
                                              name="vT")
                            nc.vector.tensor_copy(vT[:], pT[0:C, 0:32])
                    nc.sync.dma_start(
                        ohd[t0:t0 + BK, :, :].rearrange("t p c -> p t c"),
                        ohr[:].rearrange("p (s c) -> p s c", c=CP))
            # ---------- extract tags ----------            # ---------- extract tags ----------            # ---------- extract tags ----------
            def extract():
                ohd = dt['oh_d'][:, :, :].rearrange("t b c -> (t b) c")
                NJT = T // 4
                JC = min(16, NJT)
                for jc in range(NJT // JC):
                    ohch = vitpool.tile([128, JC * CP], F32, tag="ohch", bufs=2)
                    nc.sync.dma_start(
                        ohch[:].rearrange("p (j c) -> p j c", c=CP),
                        ohd[:, :].rearrange("(j p) c -> p j c", p=128)[
                            :, jc * JC:(jc + 1) * JC, :])
                    prod = vitpool.tile([128, JC * CP], F32, tag="prod", bufs=2)
                    nc.vector.tensor_tensor(
                        prod[:].rearrange("p (j c) -> p j c", c=CP),
                        ohch[:].rearrange("p (j c) -> p j c", c=CP),
                        iotaRep[:].unsqueeze(1).broadcast_to([128, JC, CP]),
                        OP.mult)
                    tf = vitpool.tile([128, JC], F32, tag="tf")
                    nc.vector.tensor_reduce(
                        tf[:], prod[:].rearrange("p (j c) -> p j c", c=CP),
                        op=OP.add, axis=AX.X)
                    ti = vitpool.tile([128, JC], I32, tag="ti")
                    nc.vector.tensor_copy(ti[:], tf[:])
                    for tl in range(4):
                        nc.sync.dma_start(
                            dt['tags'][:, :].rearrange("b (j f) -> b j f", f=4)[
                                :, jc * JC:(jc + 1) * JC, tl],
                            ti[tl * 32:(tl + 1) * 32, :])

            def next_layer_sched(gen_for_chunk):
                """Feed one chunk per block from block 66: chunk 31-j at
                block 66+2j, chunk 32+j at 67+2j (respects hbuf dump order:
                chunk c readable once dumps of 8t-blocks <= max(2c+1,127-2c)
                are emitted, i.e. from block max(2c+2, 128-2c))."""
                s = {}
                for j in range(31):
                    s.setdefault(66 + 2 * j, []).append(gen_for_chunk(31 - j))
                    s.setdefault(67 + 2 * j, []).append(gen_for_chunk(32 + j))
                # chunks 0 and 63 drain after the loop (post-drain)
                s.setdefault(10 ** 6, [])
                return s, [gen_for_chunk(0), gen_for_chunk(63)]

            ph = 63
            if ph == 63:
                bulk_load_weights(0)
                bulk_load_weights(1)
                bulk_load_weights(2)
                # bulk0: pairs (j, 63-j); j=0,1 fully up front, rest paced
                s0 = {}
                for j in range(2):
                    for ch in (j, 63 - j):
                        for kind, op in bulk_chunk_ops(0, dt['xT'], 40, ch):
                            op()
                for j in range(2, 32):
                    s0.setdefault(2 * (j - 2), []).extend([
                        bulk_chunk_ops(0, dt['xT'], 40, j),
                        bulk_chunk_ops(0, dt['xT'], 40, 63 - j)])
                # bulk1 into L0's tail
                sb1, tail1 = next_layer_sched(
                    lambda c: bulk_chunk_ops(1, dt['hbuf0'], 128, c))
                for blk, gens in sb1.items():
                    s0.setdefault(blk, []).extend(gens)
                lstm_layer(0, dt['hbuf0'], s0, lb=4)
                for g in tail1:
                    for kind, op in g:
                        op()
                # bulk2 into L1
                sb2, tail2 = next_layer_sched(
                    lambda c: bulk_chunk_ops(2, dt['hbuf1'], 128, c))
                lstm_layer(1, dt['hbuf1'], sb2, lb=4)
                for g in tail2:
                    for kind, op in g:
                        op()
                # emissions into L2 (2 chunks/block from block 66)
                se = {}
                for j in range(31):
                    se.setdefault(66 + 2 * j, []).append(
                        emissions_chunk_ops(dt['hbuf2'], 31 - j))
                    se.setdefault(67 + 2 * j, []).append(
                        emissions_chunk_ops(dt['hbuf2'], 32 + j))
                tail_e = [emissions_chunk_ops(dt['hbuf2'], 0),
                          emissions_chunk_ops(dt['hbuf2'], 63)]
                lstm_layer(2, dt['hbuf2'], se, lb=7)
                for g in tail_e:
                    for kind, op in g:
                        op()
                viterbi_fwd()
                backtrace()
                extract()
            else:
                if ph & 1:
                    bulk_gx(0, dt['xT'], 40, BF16)
                if ph & 2:
                    lstm_layer(0, dt['hbuf0'])
                    if ph & 1:
                        bulk_gx(1, dt['hbuf0'], 128, BF16)
                    lstm_layer(1, dt['hbuf1'])
                    if ph & 1:
                        bulk_gx(2, dt['hbuf1'], 128, BF16)
                    lstm_layer(2, dt['hbuf2'])
                if ph & 4:
                    emissions(dt['hbuf2'])
                if ph & 8:
                    viterbi_fwd()
                if ph & 16:
                    backtrace()
                if ph & 32:
                    extract()

    legalize_waits(nc)
    return nc


def make_in_map(inputs, cid, T, wd):
    m = {'xT': shard_x(inputs['x'], cid, T)}
    m.update(wd)
    return m


_CACHE = {}


def kernel(x, w_ih_l0, w_hh_l0, b_l0, w_ih_r, w_hh_r, b_r,
           lin_w, lin_b, crf_start, crf_end, crf_trans):
    """Full BiLSTM-CRF on 8 NeuronCores, data-parallel over the batch."""
    from concourse.bass_utils import run_bass_kernel_spmd
    T = 1024
    if 'nc' not in _CACHE:
        _CACHE['nc'] = build_nc(T)
    nc = _CACHE['nc']
    wd = prep_weights(w_ih_l0, w_hh_l0, b_l0, w_ih_r, w_hh_r, b_r,
                      lin_w, lin_b, crf_start, crf_end, crf_trans)
    in_maps = []
    for cid in range(NCORES):
        m = {'xT': shard_x(x, cid, T)}
        m.update(wd)
        in_maps.append(m)
    res = run_bass_kernel_spmd(nc, in_maps, core_ids=list(range(NCORES)))
    tags = np.concatenate([res.results[c]['tags'] for c in range(NCORES)], axis=0)
    return np.ascontiguousarray(tags.astype(np.int32))



# revision 59
# speedup vs baseline: 1.0054x; 1.0015x over previous
"""Development version of the full-device BiLSTM-CRF kernel. See design notes.

Layouts (per core, BL=32 sequences):
 - LSTM gate-major: partitions = [fwd feat 64; bwd feat 64]; psum free =
   (pair-parity, gate, batch32). Two 16-seq groups pipeline the step chain.
 - gx bulk-matmul'd (f32r/bf16, N=512) into DRAM per direction; identity
   matmul accumulates into PSUM per step pair.
 - Viterbi forward: cp sharded 4-way across partition groups; score/e/onehot
   histories time-folded [128, T/4 * 41] (partition group = t%4).
 - Backtrace: onehot chain via PE matmul with trans^T, TTR fused add+max.
"""
import sys
sys.path.insert(0, '/opt/trn_rl_repo')
import numpy as np
import ml_dtypes
import concourse.bass as bass
import concourse.mybir as mybir
from concourse.tile import TileContext

F32 = mybir.dt.float32
F32R = mybir.dt.float32  # f32r reverted: interp models f32r with reduced precision
BF16 = mybir.dt.float32  # precision experiment: all-f32
I32 = mybir.dt.int32
AF = mybir.ActivationFunctionType
OP = mybir.AluOpType
AX = mybir.AxisListType

B, D_IN, HID, C = 256, 39, 128, 41
H = HID // 2
G4 = 4 * H
NCORES = 8
BL = B // NCORES
CP = 44
NG = 4
CW = 11
NEG = -1.0e30


def legalize_waits(nc):
    n = 0
    for _, bbw in nc.bb_map.items():
        il = bbw.bb.instructions
        out = []
        for i in il:
            si = getattr(i, 'sync_info', None)
            ow = list(si.on_wait) if (si is not None and si.on_wait) else []
            if len(ow) > 1:
                for w in ow[:-1]:
                    n += 1
                    es = mybir.InstEventSemaphore(
                        name=f"legwait-{n}-{i.name}", engine=i.engine, ins=[], outs=[],
                        sync_info=mybir.SyncInfo(on_wait=[w], on_update=[]))
                    out.append(es)
                i.sync_info = mybir.SyncInfo(on_wait=[ow[-1]], on_update=list(si.on_update or []))
            out.append(i)
        bbw.bb.instructions = out
    return n


def prep_weights(w_ih_l0, w_hh_l0, b_l0, w_ih_r, w_hh_r, b_r,
                 lin_w, lin_b, crf_start, crf_end, crf_trans):
    """Gate order i,f,g,o. g rows scaled x2 (tanh(z) = 2*sigmoid(2z)-1)."""
    d = {}

    def gscale(m):
        m = np.asarray(m, np.float32).copy()
        m[2 * H:3 * H] *= 2.0
        return m

    for di, nm in ((0, 'f'), (1, 'b')):
        w = gscale(w_ih_l0[di])
        bb = gscale(b_l0[di])
        d[f'wx0_{nm}'] = np.concatenate([w.T, bb[None, :]], 0).astype(np.float32)
    for li in (0, 1):
        for di, nm in ((0, 'f'), (1, 'b')):
            w = gscale(w_ih_r[li, di])
            bb = gscale(b_r[li, di])
            d[f'wx{li+1}_{nm}'] = np.ascontiguousarray(w.T).astype(np.float32)
            d[f'bias{li+1}_{nm}'] = bb[None, :].astype(np.float32)
    for li in range(3):
        whh = np.asarray(w_hh_l0) if li == 0 else np.asarray(w_hh_r[li - 1])
        for gi in range(4):
            blk = np.zeros((128, 128), np.float32)
            sc = 2.0 if gi == 2 else 1.0
            blk[0:64, 0:64] = sc * whh[0, gi * H:(gi + 1) * H, :].T
            blk[64:128, 64:128] = sc * whh[1, gi * H:(gi + 1) * H, :].T
            d[f'whh{li}_{gi}'] = blk.astype(np.float32)
    d['ident128'] = np.eye(128, dtype=np.float32)
    d['ident16'] = np.eye(16, dtype=np.float32)
    d['ident32'] = np.eye(32, dtype=np.float32)
    d['ident44'] = np.eye(CP, dtype=np.float32)
    lw = np.zeros((HID, CP), np.float32)
    lw[:, :C] = np.asarray(lin_w, np.float32).T
    d['linWT'] = lw.astype(np.float32)
    lb = np.full((CP, 1), NEG, np.float32)
    lb[:C, 0] = np.asarray(lin_b, np.float32)
    d['linB'] = lb
    tr = np.asarray(crf_trans, np.float32)
    # transB_cn[p=(b,g), (ci, cp)] = trans[cp, g*CW+ci], NEG for pads
    transB = np.full((128, CW, CP), NEG, np.float32)
    for g in range(NG):
        for ci in range(CW):
            cn = g * CW + ci
            if cn < C:
                for b in range(32):
                    transB[b * 4 + g, ci, :C] = tr[:, cn]
    d['transB'] = transB.reshape(128, CW * CP)
    trT = np.full((C, CP), NEG, np.float32)
    trT[:, :C] = tr.T  # [cn, cp]
    d['transT'] = trT
    d['transThi'] = np.ascontiguousarray(trT[32:41])  # cn 32..40 rows
    st = np.full((128, CP), NEG, np.float32)
    st[:, :C] = np.asarray(crf_start, np.float32)[None, :]
    d['startRep'] = st
    en = np.full((32, CP), NEG, np.float32)
    en[:, :C] = np.asarray(crf_end, np.float32)[None, :]
    d['endRep'] = en
    io = np.zeros((128, CP), np.float32)
    io[:, :C] = np.arange(C, dtype=np.float32)[None, :]
    d['iotaRep'] = io
    d['onesrow'] = np.ones((1, 512), np.float32)
    gG = np.zeros((128, 4, 128), np.float32)
    for b in range(32):
        for g in range(4):
            for gp in range(4):
                gG[b * 4 + g, g, b * 4 + gp] = 1.0
    d['gatherG'] = gG.reshape(128, 512)
    d['zeros16'] = np.zeros((128, 16), np.float32)
    return d


def shard_x(x, cid, T):
    xs = np.asarray(x, np.float32)[cid * BL:(cid + 1) * BL, :T]
    xt = np.empty((D_IN + 1, T * BL), np.float32)
    xt[D_IN] = 1.0
    xt[:D_IN] = xs.transpose(2, 1, 0).reshape(D_IN, T * BL)
    return xt.astype(np.float32)


def build_nc(T):
    R = BL * T
    TJ = T // 4
    NCH = R // 512
    nc = bass.Bass()
    dt = {}

    def din(name, shape, dty=F32):
        dt[name] = nc.dram_tensor(name, shape, dty, kind="ExternalInput")

    din('xT', [D_IN + 1, R], F32R)
    din('wx0_f', [40, 256], F32R); din('wx0_b', [40, 256], F32R)
    for li in (1, 2):
        for nm in ('f', 'b'):
            din(f'wx{li}_{nm}', [128, 256], F32R)
            din(f'bias{li}_{nm}', [1, 256], F32R)
    for li in range(3):
        for gi in range(4):
            din(f'whh{li}_{gi}', [128, 128], F32)
    din('ident128', [128, 128], BF16); din('ident16', [16, 16]); din('ident32', [32, 32]); din('ident44', [CP, CP])
    din('linWT', [HID, CP], F32R); din('linB', [CP, 1])
    din('transB', [128, CW * CP]); din('transT', [C, CP])
    din('transThi', [9, CP])
    din('startRep', [128, CP]); din('endRep', [32, CP]); din('iotaRep', [128, CP])
    din('onesrow', [1, 512], F32R); din('zeros16', [128, 16], F32)
    din('gatherG', [128, 512])

    def scratch(name, shape, dty=F32):
        dt[name] = nc.dram_tensor(name, shape, dty, kind="Internal")

    for li3 in range(3):
        scratch(f'gx{li3}_f', [64, T * 128], BF16)
        scratch(f'gx{li3}_b', [64, T * 128], BF16)
    scratch('hbuf0', [HID, R], F32R)
    scratch('hbuf1', [HID, R], F32R)
    scratch('hbuf2', [HID, R], F32R)
    scratch('e_d', [T, 128, CP])
    scratch('score_d', [T, 128, CP])
    scratch('oh_d', [T, 32, CP])
    dt['tags'] = nc.dram_tensor('tags', [BL, T], I32, kind="ExternalOutput")

    with TileContext(nc) as tc:
        with tc.tile_pool(name="const", bufs=1) as cpool, \
             tc.tile_pool(name="wpool", bufs=1) as wpool, \
             tc.tile_pool(name="hist", bufs=1) as hpool, \
             tc.tile_pool(name="bulk_rhs", bufs=4) as rhspool, \
             tc.tile_pool(name="gx", bufs=8) as gxpool, \
             tc.tile_pool(name="psum", bufs=2, space="PSUM") as pspool, \
             tc.tile_pool(name="sig", bufs=8) as sigpool, \
             tc.tile_pool(name="hc", bufs=8) as hcpool, \
             tc.tile_pool(name="vit", bufs=4) as vitpool, \
             tc.tile_pool(name="emis", bufs=2) as epool:

            def load_const(nm, shape, dty=F32):
                t = cpool.tile(shape, dty, tag=nm)
                nc.sync.dma_start(t[:], dt[nm][:])
                return t

            ident128 = load_const('ident128', [128, 128], BF16)
            ident16 = load_const('ident16', [16, 16])
            ident32 = load_const('ident32', [32, 32])
            ident44 = load_const('ident44', [CP, CP])
            linWT = load_const('linWT', [HID, CP], F32R)
            linB = load_const('linB', [CP, 1])
            transB = load_const('transB', [128, CW * CP])
            transT = load_const('transT', [C, CP])
            transThi = load_const('transThi', [9, CP])
            startRep = load_const('startRep', [128, CP])
            endRep = load_const('endRep', [32, CP])
            iotaRep = load_const('iotaRep', [128, CP])
            gatherG = load_const('gatherG', [128, 512])
            whh = {}
            for li in range(3):
                for gi in range(4):
                    whh[(li, gi)] = load_const(f'whh{li}_{gi}', [128, 128], F32)
            onesrow = load_const('onesrow', [1, 512], F32R)
            zeros16 = load_const('zeros16', [128, 16], F32)

            scoreRep = hpool.tile([128, CP], F32, tag="scoreRep")

            # ---------- bulk gx (micro-op generator for interleaving) ----------
            bulk_state = {}

            def bulk_load_weights(li):
                wx = {}
                bias = {}
                for nm in ('f', 'b'):
                    wx[nm] = wpool.tile([40 if li == 0 else 128, 256], F32R,
                                        tag=f"wx{li}_{nm}", name=f"wx{li}{nm}")
                    nc.sync.dma_start(wx[nm][:], dt[f'wx{li}_{nm}'][:])
                    if li > 0:
                        bias[nm] = wpool.tile([1, 256], F32R, tag=f"bias{li}_{nm}", name=f"bias{li}{nm}")
                        nc.sync.dma_start(bias[nm][:], dt[f'bias{li}_{nm}'][:])
                bulk_state[li] = (wx, bias)

            def bulk_chunk_ops(li, src_dram, src_k, ch):
                """Yield micro-closures; caller drains them spread over time."""
                wx, bias = bulk_state[li]
                rhs = rhspool.tile([src_k, 512], F32R, tag=f"rhs{li}")
                yield ('l', lambda: nc.sync.dma_start(
                    rhs[:], src_dram[:, ch * 512:(ch + 1) * 512]))
                for nm in ('f', 'b'):
                    for pr in range(2):
                        ps = pspool.tile([128, 512], F32, tag="big", name="bps")
                        # matmuls split into 256-col halves so each drained
                        # piece fits the PE idle window of one LSTM step
                        for hh in range(4):
                            sl = slice(hh * 128, (hh + 1) * 128)
                            if li == 0:
                                yield ('h', lambda ps=ps, nm=nm, pr=pr, sl=sl:
                                       nc.tensor.matmul(
                                    ps[:, sl], wx[nm][:, pr * 128:(pr + 1) * 128],
                                    rhs[:, sl], start=True, stop=True,
                                    skip_group_check=True))
                            else:
                                yield ('h', lambda ps=ps, nm=nm, pr=pr, sl=sl:
                                       nc.tensor.matmul(
                                    ps[:, sl], wx[nm][:, pr * 128:(pr + 1) * 128],
                                    rhs[:, sl], start=True, stop=False,
                                    skip_group_check=True))
                                yield ('h', lambda ps=ps, nm=nm, pr=pr, sl=sl:
                                       nc.tensor.matmul(
                                    ps[:, sl], bias[nm][:, pr * 128:(pr + 1) * 128],
                                    onesrow[:, sl], start=False, stop=True,
                                    skip_group_check=True))
                        stg = rhspool.tile([128, 512], BF16, tag="gxstg",
                                           name="gxstg")
                        if (ch + pr) % 2 == 0:
                            yield ('l', lambda ps=ps, stg=stg: nc.scalar.activation(
                                stg[:], ps[:], AF.Copy))
                        else:
                            yield ('l', lambda ps=ps, stg=stg: nc.vector.tensor_copy(
                                stg[:], ps[:]))
                        gxd = dt[f'gx{li}_{nm}']
                        t0c = ch * 16
                        for gl in range(2):
                            gi4 = pr * 2 + gl
                            yield ('l', lambda stg=stg, gxd=gxd, t0c=t0c, gi4=gi4, gl=gl: \
                                nc.sync.dma_start(
                                    gxd[:, :].rearrange("p (t g b) -> p t g b",
                                                        g=4, b=32)[
                                        :, t0c:t0c + 16, gi4, :],
                                    stg[gl * 64:(gl + 1) * 64, :].rearrange(
                                        "p (t b) -> p t b", b=32)))

            def bulk_gx(li, src_dram, src_k, rhs_dty):
                bulk_load_weights(li)
                for ch in range(NCH):
                    for kind, op in bulk_chunk_ops(li, src_dram, src_k, ch):
                        op()

            # ---------- LSTM recurrence (skewed dual-chain pipeline) ----------
            def lstm_layer(li, hbuf_out, sched=None, lb=3):
                """sched: dict block_idx -> list of micro-op generators; ops
                drain in order, <=1 heavy + <=lb light per step, at iter end."""
                from collections import deque
                pending = deque()

                def drain(hb, lb):
                    while pending:
                        kind, fn = pending[0]
                        if kind == 'h':
                            if hb <= 0:
                                break
                            hb -= 1
                        else:
                            if lb <= 0:
                                break
                            lb -= 1
                        pending.popleft()
                        fn()

                gxf, gxb = dt[f'gx{li}_f'], dt[f'gx{li}_b']
                NB = T // 8
                cts = {}
                hprev = {}
                for g2 in range(2):
                    cts[g2] = hcpool.tile([128, 16], F32, tag=f"c{g2}", name=f"c{g2}")
                    nc.vector.memset(cts[g2][:], 0.0)
                    hprev[g2] = zeros16
                gxt = {}
                hring = {}
                pss = {}
                sig = {}
                ths = {}

                def load_block(blk):
                    t0 = blk * 8
                    g = gxpool.tile([128, 8 * 128], BF16, tag="gx")
                    nc.sync.dma_start(g[0:64, :], gxf[:, t0 * 128:(t0 + 8) * 128])
                    # bwd: reversed-t read so slot k holds t = T-1-t0-k
                    nc.sync.dma_start(
                        g[64:128, :].rearrange("p (s f) -> p s f", f=128),
                        gxb[:, :].rearrange("p (t f) -> p t f", f=128)[
                            :, T - 1 - t0:T - 9 - t0 if T - 9 - t0 >= 0 else None:-1, :])
                    gxt[blk] = g
                    hring[blk] = {
                        g2: hcpool.tile([128, 8 * 16], F32, tag=f"hr{g2}",
                                        name=f"hr{g2}") for g2 in range(2)}

                def dump_block(blk):
                    t0 = blk * 8
                    for g2 in range(2):
                        bs = g2 * 16
                        hr = hring[blk][g2]
                        nc.sync.dma_start(
                            hbuf_out[0:64, :].rearrange("p (t b) -> p t b", b=BL)[
                                :, t0:t0 + 8, bs:bs + 16],
                            hr[0:64, :].rearrange("p (s b) -> p s b", b=16))
                        nc.sync.dma_start(
                            hbuf_out[64:128, :].rearrange("p (t b) -> p t b", b=BL)[
                                :, T - 1 - t0:T - 9 - t0 if T - 9 - t0 >= 0 else None:-1,
                                bs:bs + 16],
                            hr[64:128, :].rearrange("p (s b) -> p s b", b=16))
                    del gxt[blk], hring[blk]

                def S1(g2, k):     # PE: inject gx + accumulate whh gates
                    blk, kk = divmod(k, 8)
                    bs = g2 * 16
                    ps = pspool.tile([128, 64], F32, tag=f"lps{g2}",
                                     name=f"lps{g2}")
                    gxt_v = gxt[blk][:].rearrange("p (s g b) -> p s g b",
                                                  g=4, b=32)
                    nc.tensor.matmul(
                        ps[:].rearrange("p (g b) -> p g b", g=4),
                        ident128[:], gxt_v[:, kk, :, bs:bs + 16],
                        start=True, stop=False)
                    for gi in range(4):
                        nc.tensor.matmul(
                            ps[:, gi * 16:(gi + 1) * 16],
                            whh[(li, gi)][:], hprev[g2][:],
                            start=False, stop=(gi == 3), skip_group_check=True)
                    pss[g2] = ps

                def S2(g2, k):     # Act: all-gate sigmoid
                    s = sigpool.tile([128, 64], F32, tag=f"sig{g2}",
                                     name=f"sig{g2}")
                    nc.scalar.activation(s[:], pss[g2][:], AF.Sigmoid)
                    sig[g2] = s

                def S3(g2, k):     # DVE+Pool: cell-state update
                    s = sig[g2]
                    A = sigpool.tile([128, 16], F32, tag=f"A{g2}", name=f"A{g2}")
                    nc.vector.tensor_tensor(A[:], s[:, 0:16], s[:, 32:48],
                                            OP.mult)
                    Bt = sigpool.tile([128, 16], F32, tag=f"B{g2}", name=f"B{g2}")
                    nc.vector.scalar_tensor_tensor(Bt[:], A[:], 2.0, s[:, 0:16],
                                                   OP.mult, OP.subtract)
                    Ct = sigpool.tile([128, 16], F32, tag=f"C{g2}", name=f"C{g2}")
                    nc.gpsimd.tensor_tensor(Ct[:], s[:, 16:32], cts[g2][:],
                                            OP.mult)
                    cn = hcpool.tile([128, 16], F32, tag=f"c{g2}", name=f"c{g2}")
                    nc.vector.tensor_tensor(cn[:], Bt[:], Ct[:], OP.add)
                    cts[g2] = cn

                def S4(g2, k):     # Act: tanh(c)
                    th = sigpool.tile([128, 16], F32, tag=f"th{g2}",
                                      name=f"th{g2}")
                    nc.scalar.activation(th[:], cts[g2][:], AF.Tanh)
                    ths[g2] = th

                def S5(g2, k):     # DVE: h = o * tanh(c) into ring slot
                    blk, kk = divmod(k, 8)
                    hn = hring[blk][g2][:, kk * 16:(kk + 1) * 16]
                    nc.vector.tensor_tensor(hn, sig[g2][:, 48:64], ths[g2][:],
                                            OP.mult)
                    hprev[g2] = hn

                for k in range(T):
                    blk, kk = divmod(k, 8)
                    if kk == 0:
                        if sched:
                            for gen in sched.get(blk, []):
                                pending.extend(gen)
                        load_block(blk)
                    S1(0, k)
                    if k > 0:
                        S4(1, k - 1)
                        S5(1, k - 1)
                        if kk == 0:
                            dump_block(blk - 1)
                    S2(0, k)
                    S1(1, k)
                    S3(0, k)
                    S2(1, k)
                    S4(0, k)
                    S3(1, k)
                    S5(0, k)
                    drain(4, lb)
                S4(1, T - 1)
                S5(1, T - 1)
                dump_block(NB - 1)
                drain(10 ** 9, 10 ** 9)

            # ---------- emissions (micro-op generator) ----------
            def emissions_chunk_ops(hsrc, ch):
                rhs = rhspool.tile([128, 512], F32R, tag="erhs")
                yield ('l', lambda: nc.sync.dma_start(
                    rhs[:], hsrc[:, ch * 512:(ch + 1) * 512]))
                psb = pspool.tile([128, 512], F32, tag="big", name="epsb")
                ps = psb[0:CP, :]
                for hh in range(4):
                    sl = slice(hh * 128, (hh + 1) * 128)
                    yield ('h', lambda sl=sl: nc.tensor.matmul(
                        ps[:, sl], linWT[:], rhs[:, sl],
                        start=True, stop=True, skip_group_check=True))
                eo = epool.tile([CP, 512], F32, tag="eo")
                yield ('l', lambda: nc.scalar.activation(eo[:], ps, AF.Identity,
                                                         bias=linB[:]))
                for k in range(4):
                    psTb = pspool.tile([128, 64], F32, tag="epsT", name="psTb", bufs=1)
                    psT = psTb[:, 0:CP]
                    yield ('l', lambda psT=psT, k=k: nc.tensor.transpose(
                        psT, eo[:, k * 128:(k + 1) * 128], ident44[:]))
                    estg = epool.tile([128, CP], F32, tag="estg")
                    if k % 2 == 0:
                        yield ('l', lambda psT=psT, estg=estg: \
                               nc.scalar.activation(estg[:], psT, AF.Copy))
                    else:
                        yield ('l', lambda psT=psT, estg=estg: \
                               nc.vector.tensor_copy(estg[:], psT))
                    tb = ch * 4 + k
                    ed = dt['e_d'][:, :, :].rearrange("t p c -> (t p) c")
                    yield ('l', lambda estg=estg, tb=tb, ed=ed: \
                           nc.sync.dma_start(
                        ed[:, :].rearrange("(t b q) c -> (t b) q c", b=32, q=4)[
                            tb * 128:(tb + 1) * 128, :, :],
                        estg[:].unsqueeze(1).broadcast_to([128, 4, CP])))

            def emissions(hsrc):
                for ch in range(NCH):
                    for kind, op in emissions_chunk_ops(hsrc, ch):
                        op()
            # ---------- viterbi forward ----------            # ---------- viterbi forward ----------
            def viterbi_fwd():
                ed = dt['e_d'][:, :, :]
                sd = dt['score_d'][:, :, :]
                transB_v = transB[:].rearrange("p (w c) -> p w c", c=CP)
                scoreR = {}
                et = {}
                pend_copy = None
                prev_ps = None
                nblk = T // 4
                for bk in range(nblk):
                    t0 = bk * 4
                    et[bk] = vitpool.tile([128, 4 * CP], F32, tag="ein", name="ein")
                    nc.sync.dma_start(
                        et[bk][:].rearrange("p (s c) -> p s c", c=CP),
                        ed[t0:t0 + 4].rearrange("t p c -> p t c"))
                    scoreR[bk] = vitpool.tile([128, 4 * CP], F32, tag="sring",
                                              name="sring")
                    for k in range(4):
                        t = t0 + k
                        e_sl = et[bk][:, k * CP:(k + 1) * CP]
                        out_sl = scoreR[bk][:, k * CP:(k + 1) * CP]
                        if t == 0:
                            nc.vector.tensor_tensor(out_sl, startRep[:], e_sl, OP.add)
                            continue
                        prev = scoreR[bk][:, 0:CP] if t == 1 else \
                            prev_ps[:, 0:CP]
                        cand = vitpool.tile([128, CW * C], F32, tag="cand")
                        cand_v = cand[:].rearrange("p (w c) -> p w c", c=C)
                        nc.vector.tensor_tensor(
                            cand_v,
                            prev[:, 0:C].unsqueeze(1).broadcast_to([128, CW, C]),
                            transB_v[:, :, 0:C], OP.add)
                        bsh = vitpool.tile([128, CW], F32, tag="bsh")
                        nc.vector.tensor_reduce(
                            bsh[:], cand_v, op=OP.max, axis=AX.X)
                        # score(t) = best + e built fully in PSUM: e seeded by
                        # an identity mm (start=True), then 4 gather mms
                        # accumulate the distributed best (single group).
                        # The chain reads score straight from PSUM.
                        psc = pspool.tile([128, 64], F32, tag="lps0",
                                          name="vfps")
                        nc.tensor.matmul(psc[:, 0:CP], ident128[:], e_sl,
                                         start=True, stop=False,
                                         skip_group_check=True)
                        for g in range(4):
                            nc.tensor.matmul(
                                psc[:, g * CW:(g + 1) * CW],
                                gatherG[:, g * 128:(g + 1) * 128], bsh[:],
                                start=False, stop=(g == 3),
                                skip_group_check=True)
                        # history copy (Act, off-chain), deferred one step so
                        # tile-granular tracking can't stall the next cand
                        if pend_copy is not None:
                            nc.scalar.activation(pend_copy[0], pend_copy[1],
                                                 AF.Copy)
                            pend_copy = None
                        if k == 0 and bk >= 1:
                            nc.sync.dma_start(
                                sd[t0 - 4:t0].rearrange("t p c -> p t c"),
                                scoreR[bk - 1][:].rearrange(
                                    "p (s c) -> p s c", c=CP))
                        pend_copy = (out_sl, psc[:, 0:CP])
                        prev_ps = psc
                    if bk == nblk - 1:
                        if pend_copy is not None:
                            nc.scalar.activation(pend_copy[0], pend_copy[1],
                                                 AF.Copy)
                            pend_copy = None
                        nc.sync.dma_start(
                            sd[t0:t0 + 4].rearrange("t p c -> p t c"),
                            scoreR[bk][:].rearrange("p (s c) -> p s c", c=CP))
                    if bk >= 2:
                        del scoreR[bk - 2], et[bk - 2]
            # ---------- backtrace ----------            # ---------- backtrace ----------            # ---------- backtrace ----------
            def backtrace():
                # Single 32-wide chain: per step 1 PE mm (pu = vT @ transT) +
                # add + max-reduce + is_ge + PE transpose + Act copy (vT).
                sd = dt['score_d'][:, :, :]
                ohd = dt['oh_d'][:, :, :]
                BK = 8
                vT = None
                for bk in range(T // BK - 1, -1, -1):
                    t0 = bk * BK   # block covers t0 .. t0+7
                    sc = vitpool.tile([32, BK * CP], F32, tag="sc", name="sc")
                    nc.sync.dma_start(
                        sc[:].rearrange("p (s c) -> p s c", c=CP),
                        sd[t0:t0 + BK].rearrange(
                            "t (b q) c -> b t q c", q=4)[:, :, 0, :])
                    ohr = vitpool.tile([32, BK * CP], F32, tag="ohr", name="ohr")
                    ur = vitpool.tile([32, BK * CP], F32, tag="ur", name="ur")
                    for k in range(BK - 1, -1, -1):
                        t = t0 + k
                        sc_sl = sc[:, k * CP:(k + 1) * CP]
                        u_sl = ur[:, k * CP:(k + 1) * CP]
                        m2 = vitpool.tile([32, 1], F32, tag="m2", name="m2")
                        if t == T - 1:
                            nc.vector.tensor_tensor(u_sl, sc_sl, endRep[:],
                                                    OP.add)
                        else:
                            pub = pspool.tile([64, 64], F32, tag="bt",
                                              name="pub", bufs=1)
                            pu = pub[0:32, 0:CP]
                            nc.tensor.matmul(pu, vT[:], transT[:],
                                             start=True, stop=True)
                            nc.vector.tensor_tensor(u_sl, sc_sl, pu, OP.add)
                        nc.vector.tensor_reduce(m2[:], u_sl, op=OP.max,
                                                axis=AX.X)
                        oh_sl = ohr[:, k * CP:(k + 1) * CP]
                        nc.vector.tensor_tensor(
                            oh_sl, u_sl, m2[:].broadcast_to([32, CP]),
                            OP.is_ge)
                        if t > 0:
                            pT = pspool.tile([64, 64], F32, tag="bt",
                                             name="pT", bufs=1)
                            nc.tensor.transpose(pT[0:CP, 0:32], oh_sl, ident32[:])
                            vT = sigpool.tile([C, 32], F32, tag="vT",
                                              name="vT")
                            nc.vector.tensor_copy(vT[:], pT[0:C, 0:32])
                    nc.sync.dma_start(
                        ohd[t0:t0 + BK, :, :].rearrange("t p c -> p t c"),
                        ohr[:].rearrange("p (s c) -> p s c", c=CP))
            # ---------- extract tags ----------            # ---------- extract tags ----------            # ---------- extract tags ----------
            def extract():
                ohd = dt['oh_d'][:, :, :].rearrange("t b c -> (t b) c")
                NJT = T // 4
                JC = min(16, NJT)
                for jc in range(NJT // JC):
                    ohch = vitpool.tile([128, JC * CP], F32, tag="ohch", bufs=2)
                    nc.sync.dma_start(
                        ohch[:].rearrange("p (j c) -> p j c", c=CP),
                        ohd[:, :].rearrange("(j p) c -> p j c", p=128)[
                            :, jc * JC:(jc + 1) * JC, :])
                    prod = vitpool.tile([128, JC * CP], F32, tag="prod", bufs=2)
                    nc.vector.tensor_tensor(
                        prod[:].rearrange("p (j c) -> p j c", c=CP),
                        ohch[:].rearrange("p (j c) -> p j c", c=CP),
                        iotaRep[:].unsqueeze(1).broadcast_to([128, JC, CP]),
                        OP.mult)
                    tf = vitpool.tile([128, JC], F32, tag="tf")
                    nc.vector.tensor_reduce(
                        tf[:], prod[:].rearrange("p (j c) -> p j c", c=CP),
                        op=OP.add, axis=AX.X)
                    ti = vitpool.tile([128, JC], I32, tag="ti")
                    nc.vector.tensor_copy(ti[:], tf[:])
                    for tl in range(4):
                        nc.sync.dma_start(
                            dt['tags'][:, :].rearrange("b (j f) -> b j f", f=4)[
                                :, jc * JC:(jc + 1) * JC, tl],
                            ti[tl * 32:(tl + 1) * 32, :])

            def next_layer_sched(gen_for_chunk):
                """Feed one chunk per block from block 66: chunk 31-j at
                block 66+2j, chunk 32+j at 67+2j (respects hbuf dump order:
                chunk c readable once dumps of 8t-blocks <= max(2c+1,127-2c)
                are emitted, i.e. from block max(2c+2, 128-2c))."""
                s = {}
                for j in range(31):
                    s.setdefault(66 + 2 * j, []).append(gen_for_chunk(31 - j))
                    s.setdefault(67 + 2 * j, []).append(gen_for_chunk(32 + j))
                # chunks 0 and 63 drain after the loop (post-drain)
                s.setdefault(10 ** 6, [])
                return s, [gen_for_chunk(0), gen_for_chunk(63)]

            ph = 63
            if ph == 63:
                bulk_load_weights(0)
                bulk_load_weights(1)
                bulk_load_weights(2)
                # bulk0: pairs (j, 63-j); j=0,1 fully up front, rest paced
                s0 = {}
                for j in range(2):
                    for ch in (j, 63 - j):
                        for kind, op in bulk_chunk_ops(0, dt['xT'], 40, ch):
                            op()
                for j in range(2, 32):
                    s0.setdefault(2 * (j - 2), []).extend([
                        bulk_chunk_ops(0, dt['xT'], 40, j),
                        bulk_chunk_ops(0, dt['xT'], 40, 63 - j)])
                # bulk1 into L0's tail
                sb1, tail1 = next_layer_sched(
                    lambda c: bulk_chunk_ops(1, dt['hbuf0'], 128, c))
                for blk, gens in sb1.items():
                    s0.setdefault(blk, []).extend(gens)
                lstm_layer(0, dt['hbuf0'], s0, lb=4)
                for g in tail1:
                    for kind, op in g:
                        op()
                # bulk2 into L1
                sb2, tail2 = next_layer_sched(
                    lambda c: bulk_chunk_ops(2, dt['hbuf1'], 128, c))
                lstm_layer(1, dt['hbuf1'], sb2, lb=4)
                for g in tail2:
                    for kind, op in g:
                        op()
                # emissions into L2 (2 chunks/block from block 66)
                se = {}
                for j in range(31):
                    se.setdefault(66 + 2 * j, []).append(
                        emissions_chunk_ops(dt['hbuf2'], 31 - j))
                    se.setdefault(67 + 2 * j, []).append(
                        emissions_chunk_ops(dt['hbuf2'], 32 + j))
                tail_e = [emissions_chunk_ops(dt['hbuf2'], 0),
                          emissions_chunk_ops(dt['hbuf2'], 63)]
                lstm_layer(2, dt['hbuf2'], se, lb=7)
                for g in tail_e:
                    for kind, op in g:
                        op()
                viterbi_fwd()
                backtrace()
                extract()
            else:
                if ph & 1:
                    bulk_gx(0, dt['xT'], 40, BF16)
                if ph & 2:
                    lstm_layer(0, dt['hbuf0'])
                    if ph & 1:
                        bulk_gx(1, dt['hbuf0'], 128, BF16)
                    lstm_layer(1, dt['hbuf1'])
                    if ph & 1:
                        bulk_gx(2, dt['hbuf1'], 128, BF16)
                    lstm_layer(2, dt['hbuf2'])
                if ph & 4:
                    emissions(dt['hbuf2'])
                if ph & 8:
                    viterbi_fwd()
                if ph & 16:
                    backtrace()
                if ph & 32:
                    extract()

    legalize_waits(nc)
    return nc


def make_in_map(inputs, cid, T, wd):
    m = {'xT': shard_x(inputs['x'], cid, T)}
    m.update(wd)
    return m


_CACHE = {}


def kernel(x, w_ih_l0, w_hh_l0, b_l0, w_ih_r, w_hh_r, b_r,
           lin_w, lin_b, crf_start, crf_end, crf_trans):
    """Full BiLSTM-CRF on 8 NeuronCores, data-parallel over the batch."""
    from concourse.bass_utils import run_bass_kernel_spmd
    T = 1024
    if 'nc' not in _CACHE:
        _CACHE['nc'] = build_nc(T)
    nc = _CACHE['nc']
    wd = prep_weights(w_ih_l0, w_hh_l0, b_l0, w_ih_r, w_hh_r, b_r,
                      lin_w, lin_b, crf_start, crf_end, crf_trans)
    in_maps = []
    for cid in range(NCORES):
        m = {'xT': shard_x(x, cid, T)}
        m.update(wd)
        in_maps.append(m)
    res = run_bass_kernel_spmd(nc, in_maps, core_ids=list(range(NCORES)))
    tags = np.concatenate([res.results[c]['tags'] for c in range(NCORES)], axis=0)
    return np.ascontiguousarray(tags.astype(np.int32))



# revision 61
# speedup vs baseline: 1.0098x; 1.0045x over previous
"""Development version of the full-device BiLSTM-CRF kernel. See design notes.

Layouts (per core, BL=32 sequences):
 - LSTM gate-major: partitions = [fwd feat 64; bwd feat 64]; psum free =
   (pair-parity, gate, batch32). Two 16-seq groups pipeline the step chain.
 - gx bulk-matmul'd (f32r/bf16, N=512) into DRAM per direction; identity
   matmul accumulates into PSUM per step pair.
 - Viterbi forward: cp sharded 4-way across partition groups; score/e/onehot
   histories time-folded [128, T/4 * 41] (partition group = t%4).
 - Backtrace: onehot chain via PE matmul with trans^T, TTR fused add+max.
"""
import sys
sys.path.insert(0, '/opt/trn_rl_repo')
import numpy as np
import ml_dtypes
import concourse.bass as bass
import concourse.mybir as mybir
from concourse.tile import TileContext

F32 = mybir.dt.float32
F32R = mybir.dt.float32  # f32r reverted: interp models f32r with reduced precision
BF16 = mybir.dt.float32  # precision experiment: all-f32
I32 = mybir.dt.int32
AF = mybir.ActivationFunctionType
OP = mybir.AluOpType
AX = mybir.AxisListType

B, D_IN, HID, C = 256, 39, 128, 41
H = HID // 2
G4 = 4 * H
NCORES = 8
BL = B // NCORES
CP = 44
NG = 4
CW = 11
NEG = -1.0e30


def legalize_waits(nc):
    n = 0
    for _, bbw in nc.bb_map.items():
        il = bbw.bb.instructions
        out = []
        for i in il:
            si = getattr(i, 'sync_info', None)
            ow = list(si.on_wait) if (si is not None and si.on_wait) else []
            if len(ow) > 1:
                for w in ow[:-1]:
                    n += 1
                    es = mybir.InstEventSemaphore(
                        name=f"legwait-{n}-{i.name}", engine=i.engine, ins=[], outs=[],
                        sync_info=mybir.SyncInfo(on_wait=[w], on_update=[]))
                    out.append(es)
                i.sync_info = mybir.SyncInfo(on_wait=[ow[-1]], on_update=list(si.on_update or []))
            out.append(i)
        bbw.bb.instructions = out
    return n


def prep_weights(w_ih_l0, w_hh_l0, b_l0, w_ih_r, w_hh_r, b_r,
                 lin_w, lin_b, crf_start, crf_end, crf_trans):
    """Gate order i,f,g,o. g rows scaled x2 (tanh(z) = 2*sigmoid(2z)-1)."""
    d = {}

    def gscale(m):
        m = np.asarray(m, np.float32).copy()
        m[2 * H:3 * H] *= 2.0
        return m

    for di, nm in ((0, 'f'), (1, 'b')):
        w = gscale(w_ih_l0[di])
        bb = gscale(b_l0[di])
        d[f'wx0_{nm}'] = np.concatenate([w.T, bb[None, :]], 0).astype(np.float32)
    for li in (0, 1):
        for di, nm in ((0, 'f'), (1, 'b')):
            w = gscale(w_ih_r[li, di])
            bb = gscale(b_r[li, di])
            d[f'wx{li+1}_{nm}'] = np.ascontiguousarray(w.T).astype(np.float32)
            d[f'bias{li+1}_{nm}'] = bb[None, :].astype(np.float32)
    for li in range(3):
        whh = np.asarray(w_hh_l0) if li == 0 else np.asarray(w_hh_r[li - 1])
        for gi in range(4):
            blk = np.zeros((128, 128), np.float32)
            sc = 2.0 if gi == 2 else 1.0
            blk[0:64, 0:64] = sc * whh[0, gi * H:(gi + 1) * H, :].T
            blk[64:128, 64:128] = sc * whh[1, gi * H:(gi + 1) * H, :].T
            d[f'whh{li}_{gi}'] = blk.astype(np.float32)
    d['ident128'] = np.eye(128, dtype=np.float32)
    d['ident16'] = np.eye(16, dtype=np.float32)
    d['ident32'] = np.eye(32, dtype=np.float32)
    d['ident44'] = np.eye(CP, dtype=np.float32)
    lw = np.zeros((HID, CP), np.float32)
    lw[:, :C] = np.asarray(lin_w, np.float32).T
    d['linWT'] = lw.astype(np.float32)
    lb = np.full((CP, 1), NEG, np.float32)
    lb[:C, 0] = np.asarray(lin_b, np.float32)
    d['linB'] = lb
    tr = np.asarray(crf_trans, np.float32)
    # transB_cn[p=(b,g), (ci, cp)] = trans[cp, g*CW+ci], NEG for pads
    transB = np.full((128, CW, CP), NEG, np.float32)
    for g in range(NG):
        for ci in range(CW):
            cn = g * CW + ci
            if cn < C:
                for b in range(32):
                    transB[b * 4 + g, ci, :C] = tr[:, cn]
    d['transB'] = transB.reshape(128, CW * CP)
    trT = np.full((C, CP), NEG, np.float32)
    trT[:, :C] = tr.T  # [cn, cp]
    d['transT'] = trT
    d['transThi'] = np.ascontiguousarray(trT[32:41])  # cn 32..40 rows
    st = np.full((128, CP), NEG, np.float32)
    st[:, :C] = np.asarray(crf_start, np.float32)[None, :]
    d['startRep'] = st
    en = np.full((32, CP), NEG, np.float32)
    en[:, :C] = np.asarray(crf_end, np.float32)[None, :]
    d['endRep'] = en
    io = np.zeros((128, CP), np.float32)
    io[:, :C] = np.arange(C, dtype=np.float32)[None, :]
    d['iotaRep'] = io
    d['onesrow'] = np.ones((1, 512), np.float32)
    gG = np.zeros((128, 4, 128), np.float32)
    for b in range(32):
        for g in range(4):
            for gp in range(4):
                gG[b * 4 + g, g, b * 4 + gp] = 1.0
    d['gatherG'] = gG.reshape(128, 512)
    d['zeros16'] = np.zeros((128, 16), np.float32)
    return d


def shard_x(x, cid, T):
    xs = np.asarray(x, np.float32)[cid * BL:(cid + 1) * BL, :T]
    xt = np.empty((D_IN + 1, T * BL), np.float32)
    xt[D_IN] = 1.0
    xt[:D_IN] = xs.transpose(2, 1, 0).reshape(D_IN, T * BL)
    return xt.astype(np.float32)


def build_nc(T):
    R = BL * T
    TJ = T // 4
    NCH = R // 512
    nc = bass.Bass()
    dt = {}

    def din(name, shape, dty=F32):
        dt[name] = nc.dram_tensor(name, shape, dty, kind="ExternalInput")

    din('xT', [D_IN + 1, R], F32R)
    din('wx0_f', [40, 256], F32R); din('wx0_b', [40, 256], F32R)
    for li in (1, 2):
        for nm in ('f', 'b'):
            din(f'wx{li}_{nm}', [128, 256], F32R)
            din(f'bias{li}_{nm}', [1, 256], F32R)
    for li in range(3):
        for gi in range(4):
            din(f'whh{li}_{gi}', [128, 128], F32)
    din('ident128', [128, 128], BF16); din('ident16', [16, 16]); din('ident32', [32, 32]); din('ident44', [CP, CP])
    din('linWT', [HID, CP], F32R); din('linB', [CP, 1])
    din('transB', [128, CW * CP]); din('transT', [C, CP])
    din('transThi', [9, CP])
    din('startRep', [128, CP]); din('endRep', [32, CP]); din('iotaRep', [128, CP])
    din('onesrow', [1, 512], F32R); din('zeros16', [128, 16], F32)
    din('gatherG', [128, 512])

    def scratch(name, shape, dty=F32):
        dt[name] = nc.dram_tensor(name, shape, dty, kind="Internal")

    for li3 in range(3):
        scratch(f'gx{li3}_f', [64, T * 128], BF16)
        scratch(f'gx{li3}_b', [64, T * 128], BF16)
    scratch('hbuf0', [HID, R], F32R)
    scratch('hbuf1', [HID, R], F32R)
    scratch('hbuf2', [HID, R], F32R)
    scratch('e_d', [T, 128, CP])
    scratch('score_d', [T, 128, CP])
    scratch('oh_d', [T, 32, CP])
    dt['tags'] = nc.dram_tensor('tags', [BL, T], I32, kind="ExternalOutput")

    with TileContext(nc) as tc:
        with tc.tile_pool(name="const", bufs=1) as cpool, \
             tc.tile_pool(name="wpool", bufs=1) as wpool, \
             tc.tile_pool(name="hist", bufs=1) as hpool, \
             tc.tile_pool(name="bulk_rhs", bufs=4) as rhspool, \
             tc.tile_pool(name="gx", bufs=8) as gxpool, \
             tc.tile_pool(name="psum", bufs=2, space="PSUM") as pspool, \
             tc.tile_pool(name="sig", bufs=8) as sigpool, \
             tc.tile_pool(name="hc", bufs=8) as hcpool, \
             tc.tile_pool(name="vit", bufs=4) as vitpool, \
             tc.tile_pool(name="emis", bufs=2) as epool:

            def load_const(nm, shape, dty=F32):
                t = cpool.tile(shape, dty, tag=nm)
                nc.sync.dma_start(t[:], dt[nm][:])
                return t

            ident128 = load_const('ident128', [128, 128], BF16)
            ident16 = load_const('ident16', [16, 16])
            ident32 = load_const('ident32', [32, 32])
            ident44 = load_const('ident44', [CP, CP])
            linWT = load_const('linWT', [HID, CP], F32R)
            linB = load_const('linB', [CP, 1])
            transB = load_const('transB', [128, CW * CP])
            transT = load_const('transT', [C, CP])
            transThi = load_const('transThi', [9, CP])
            startRep = load_const('startRep', [128, CP])
            endRep = load_const('endRep', [32, CP])
            iotaRep = load_const('iotaRep', [128, CP])
            gatherG = load_const('gatherG', [128, 512])
            whh = {}
            for li in range(3):
                for gi in range(4):
                    whh[(li, gi)] = load_const(f'whh{li}_{gi}', [128, 128], F32)
            onesrow = load_const('onesrow', [1, 512], F32R)
            zeros16 = load_const('zeros16', [128, 16], F32)

            scoreRep = hpool.tile([128, CP], F32, tag="scoreRep")

            # ---------- bulk gx (micro-op generator for interleaving) ----------
            bulk_state = {}

            def bulk_load_weights(li):
                wx = {}
                bias = {}
                for nm in ('f', 'b'):
                    wx[nm] = wpool.tile([40 if li == 0 else 128, 256], F32R,
                                        tag=f"wx{li}_{nm}", name=f"wx{li}{nm}")
                    nc.sync.dma_start(wx[nm][:], dt[f'wx{li}_{nm}'][:])
                    if li > 0:
                        bias[nm] = wpool.tile([1, 256], F32R, tag=f"bias{li}_{nm}", name=f"bias{li}{nm}")
                        nc.sync.dma_start(bias[nm][:], dt[f'bias{li}_{nm}'][:])
                bulk_state[li] = (wx, bias)

            def bulk_chunk_ops(li, src_dram, src_k, ch):
                """Yield micro-closures; caller drains them spread over time."""
                wx, bias = bulk_state[li]
                rhs = rhspool.tile([src_k, 512], F32R, tag=f"rhs{li}")
                yield ('l', lambda: nc.sync.dma_start(
                    rhs[:], src_dram[:, ch * 512:(ch + 1) * 512]))
                for nm in ('f', 'b'):
                    for pr in range(2):
                        ps = pspool.tile([128, 512], F32, tag="big", name="bps")
                        # matmuls split into 256-col halves so each drained
                        # piece fits the PE idle window of one LSTM step
                        for hh in range(4):
                            sl = slice(hh * 128, (hh + 1) * 128)
                            if li == 0:
                                yield ('h', lambda ps=ps, nm=nm, pr=pr, sl=sl:
                                       nc.tensor.matmul(
                                    ps[:, sl], wx[nm][:, pr * 128:(pr + 1) * 128],
                                    rhs[:, sl], start=True, stop=True,
                                    skip_group_check=True))
                            else:
                                yield ('h', lambda ps=ps, nm=nm, pr=pr, sl=sl:
                                       nc.tensor.matmul(
                                    ps[:, sl], wx[nm][:, pr * 128:(pr + 1) * 128],
                                    rhs[:, sl], start=True, stop=False,
                                    skip_group_check=True))
                                yield ('h', lambda ps=ps, nm=nm, pr=pr, sl=sl:
                                       nc.tensor.matmul(
                                    ps[:, sl], bias[nm][:, pr * 128:(pr + 1) * 128],
                                    onesrow[:, sl], start=False, stop=True,
                                    skip_group_check=True))
                        stg = rhspool.tile([128, 512], BF16, tag="gxstg",
                                           name="gxstg")
                        if (ch + pr) % 2 == 0:
                            yield ('l', lambda ps=ps, stg=stg: nc.scalar.activation(
                                stg[:], ps[:], AF.Copy))
                        else:
                            yield ('l', lambda ps=ps, stg=stg: nc.vector.tensor_copy(
                                stg[:], ps[:]))
                        gxd = dt[f'gx{li}_{nm}']
                        t0c = ch * 16
                        for gl in range(2):
                            gi4 = pr * 2 + gl
                            yield ('l', lambda stg=stg, gxd=gxd, t0c=t0c, gi4=gi4, gl=gl: \
                                nc.sync.dma_start(
                                    gxd[:, :].rearrange("p (t g b) -> p t g b",
                                                        g=4, b=32)[
                                        :, t0c:t0c + 16, gi4, :],
                                    stg[gl * 64:(gl + 1) * 64, :].rearrange(
                                        "p (t b) -> p t b", b=32)))

            def bulk_gx(li, src_dram, src_k, rhs_dty):
                bulk_load_weights(li)
                for ch in range(NCH):
                    for kind, op in bulk_chunk_ops(li, src_dram, src_k, ch):
                        op()

            # ---------- LSTM recurrence (skewed dual-chain pipeline) ----------
            def lstm_layer(li, hbuf_out, sched=None, lb=3):
                """sched: dict block_idx -> list of micro-op generators; ops
                drain in order, <=1 heavy + <=lb light per step, at iter end."""
                from collections import deque
                pending = deque()

                def drain(hb, lb):
                    while pending:
                        kind, fn = pending[0]
                        if kind == 'h':
                            if hb <= 0:
                                break
                            hb -= 1
                        else:
                            if lb <= 0:
                                break
                            lb -= 1
                        pending.popleft()
                        fn()

                gxf, gxb = dt[f'gx{li}_f'], dt[f'gx{li}_b']
                NB = T // 8
                cts = {}
                hprev = {}
                for g2 in range(2):
                    cts[g2] = hcpool.tile([128, 16], F32, tag=f"c{g2}", name=f"c{g2}")
                    nc.vector.memset(cts[g2][:], 0.0)
                    hprev[g2] = zeros16
                gxt = {}
                hring = {}
                pss = {}
                sig = {}
                ths = {}

                def load_block(blk):
                    t0 = blk * 8
                    g = gxpool.tile([128, 8 * 128], BF16, tag="gx")
                    nc.sync.dma_start(g[0:64, :], gxf[:, t0 * 128:(t0 + 8) * 128])
                    # bwd: reversed-t read so slot k holds t = T-1-t0-k
                    nc.sync.dma_start(
                        g[64:128, :].rearrange("p (s f) -> p s f", f=128),
                        gxb[:, :].rearrange("p (t f) -> p t f", f=128)[
                            :, T - 1 - t0:T - 9 - t0 if T - 9 - t0 >= 0 else None:-1, :])
                    gxt[blk] = g
                    hring[blk] = {
                        g2: hcpool.tile([128, 8 * 16], F32, tag=f"hr{g2}",
                                        name=f"hr{g2}") for g2 in range(2)}

                def dump_block(blk):
                    t0 = blk * 8
                    for g2 in range(2):
                        bs = g2 * 16
                        hr = hring[blk][g2]
                        nc.sync.dma_start(
                            hbuf_out[0:64, :].rearrange("p (t b) -> p t b", b=BL)[
                                :, t0:t0 + 8, bs:bs + 16],
                            hr[0:64, :].rearrange("p (s b) -> p s b", b=16))
                        nc.sync.dma_start(
                            hbuf_out[64:128, :].rearrange("p (t b) -> p t b", b=BL)[
                                :, T - 1 - t0:T - 9 - t0 if T - 9 - t0 >= 0 else None:-1,
                                bs:bs + 16],
                            hr[64:128, :].rearrange("p (s b) -> p s b", b=16))
                    del gxt[blk], hring[blk]

                def S1(g2, k):     # PE: inject gx + accumulate whh gates
                    blk, kk = divmod(k, 8)
                    bs = g2 * 16
                    ps = pspool.tile([128, 64], F32, tag=f"lps{g2}",
                                     name=f"lps{g2}")
                    gxt_v = gxt[blk][:].rearrange("p (s g b) -> p s g b",
                                                  g=4, b=32)
                    nc.tensor.matmul(
                        ps[:].rearrange("p (g b) -> p g b", g=4),
                        ident128[:], gxt_v[:, kk, :, bs:bs + 16],
                        start=True, stop=False)
                    for gi in range(4):
                        nc.tensor.matmul(
                            ps[:, gi * 16:(gi + 1) * 16],
                            whh[(li, gi)][:], hprev[g2][:],
                            start=False, stop=(gi == 3), skip_group_check=True)
                    pss[g2] = ps

                def S2(g2, k):     # Act: all-gate sigmoid
                    s = sigpool.tile([128, 64], F32, tag=f"sig{g2}",
                                     name=f"sig{g2}")
                    nc.scalar.activation(s[:], pss[g2][:], AF.Sigmoid)
                    sig[g2] = s

                def S3(g2, k):     # DVE+Pool: cell-state update
                    s = sig[g2]
                    A = sigpool.tile([128, 16], F32, tag=f"A{g2}", name=f"A{g2}")
                    nc.vector.tensor_tensor(A[:], s[:, 0:16], s[:, 32:48],
                                            OP.mult)
                    Bt = sigpool.tile([128, 16], F32, tag=f"B{g2}", name=f"B{g2}")
                    nc.vector.scalar_tensor_tensor(Bt[:], A[:], 2.0, s[:, 0:16],
                                                   OP.mult, OP.subtract)
                    Ct = sigpool.tile([128, 16], F32, tag=f"C{g2}", name=f"C{g2}")
                    nc.gpsimd.tensor_tensor(Ct[:], s[:, 16:32], cts[g2][:],
                                            OP.mult)
                    cn = hcpool.tile([128, 16], F32, tag=f"c{g2}", name=f"c{g2}")
                    nc.vector.tensor_tensor(cn[:], Bt[:], Ct[:], OP.add)
                    cts[g2] = cn

                def S4(g2, k):     # Act: tanh(c)
                    th = sigpool.tile([128, 16], F32, tag=f"th{g2}",
                                      name=f"th{g2}")
                    nc.scalar.activation(th[:], cts[g2][:], AF.Tanh)
                    ths[g2] = th

                def S5(g2, k):     # DVE: h = o * tanh(c) into ring slot
                    blk, kk = divmod(k, 8)
                    hn = hring[blk][g2][:, kk * 16:(kk + 1) * 16]
                    nc.vector.tensor_tensor(hn, sig[g2][:, 48:64], ths[g2][:],
                                            OP.mult)
                    hprev[g2] = hn

                for k in range(T):
                    blk, kk = divmod(k, 8)
                    if kk == 0:
                        if sched:
                            for gen in sched.get(blk, []):
                                pending.extend(gen)
                        load_block(blk)
                    S1(0, k)
                    if k > 0:
                        S4(1, k - 1)
                        S5(1, k - 1)
                        if kk == 0:
                            dump_block(blk - 1)
                    S2(0, k)
                    S1(1, k)
                    S3(0, k)
                    S2(1, k)
                    drain(2, lb // 2)
                    S4(0, k)
                    S3(1, k)
                    S5(0, k)
                    drain(2, lb - lb // 2)
                S4(1, T - 1)
                S5(1, T - 1)
                dump_block(NB - 1)
                drain(10 ** 9, 10 ** 9)

            # ---------- emissions (micro-op generator) ----------
            def emissions_chunk_ops(hsrc, ch):
                rhs = rhspool.tile([128, 512], F32R, tag="erhs")
                yield ('l', lambda: nc.sync.dma_start(
                    rhs[:], hsrc[:, ch * 512:(ch + 1) * 512]))
                psb = pspool.tile([128, 512], F32, tag="big", name="epsb")
                ps = psb[0:CP, :]
                for hh in range(4):
                    sl = slice(hh * 128, (hh + 1) * 128)
                    yield ('h', lambda sl=sl: nc.tensor.matmul(
                        ps[:, sl], linWT[:], rhs[:, sl],
                        start=True, stop=True, skip_group_check=True))
                eo = epool.tile([CP, 512], F32, tag="eo")
                yield ('l', lambda: nc.scalar.activation(eo[:], ps, AF.Identity,
                                                         bias=linB[:]))
                for k in range(4):
                    psTb = pspool.tile([128, 64], F32, tag="epsT", name="psTb", bufs=1)
                    psT = psTb[:, 0:CP]
                    yield ('l', lambda psT=psT, k=k: nc.tensor.transpose(
                        psT, eo[:, k * 128:(k + 1) * 128], ident44[:]))
                    estg = epool.tile([128, CP], F32, tag="estg")
                    if k % 2 == 0:
                        yield ('l', lambda psT=psT, estg=estg: \
                               nc.scalar.activation(estg[:], psT, AF.Copy))
                    else:
                        yield ('l', lambda psT=psT, estg=estg: \
                               nc.vector.tensor_copy(estg[:], psT))
                    tb = ch * 4 + k
                    ed = dt['e_d'][:, :, :].rearrange("t p c -> (t p) c")
                    yield ('l', lambda estg=estg, tb=tb, ed=ed: \
                           nc.sync.dma_start(
                        ed[:, :].rearrange("(t b q) c -> (t b) q c", b=32, q=4)[
                            tb * 128:(tb + 1) * 128, :, :],
                        estg[:].unsqueeze(1).broadcast_to([128, 4, CP])))

            def emissions(hsrc):
                for ch in range(NCH):
                    for kind, op in emissions_chunk_ops(hsrc, ch):
                        op()
            # ---------- viterbi forward ----------            # ---------- viterbi forward ----------
            def viterbi_fwd():
                ed = dt['e_d'][:, :, :]
                sd = dt['score_d'][:, :, :]
                transB_v = transB[:].rearrange("p (w c) -> p w c", c=CP)
                scoreR = {}
                et = {}
                pend_copy = None
                prev_ps = None
                nblk = T // 4
                for bk in range(nblk):
                    t0 = bk * 4
                    et[bk] = vitpool.tile([128, 4 * CP], F32, tag="ein", name="ein")
                    nc.sync.dma_start(
                        et[bk][:].rearrange("p (s c) -> p s c", c=CP),
                        ed[t0:t0 + 4].rearrange("t p c -> p t c"))
                    scoreR[bk] = vitpool.tile([128, 4 * CP], F32, tag="sring",
                                              name="sring")
                    for k in range(4):
                        t = t0 + k
                        e_sl = et[bk][:, k * CP:(k + 1) * CP]
                        out_sl = scoreR[bk][:, k * CP:(k + 1) * CP]
                        if t == 0:
                            nc.vector.tensor_tensor(out_sl, startRep[:], e_sl, OP.add)
                            continue
                        prev = scoreR[bk][:, 0:CP] if t == 1 else \
                            prev_ps[:, 0:CP]
                        cand = vitpool.tile([128, CW * C], F32, tag="cand")
                        cand_v = cand[:].rearrange("p (w c) -> p w c", c=C)
                        nc.vector.tensor_tensor(
                            cand_v,
                            prev[:, 0:C].unsqueeze(1).broadcast_to([128, CW, C]),
                            transB_v[:, :, 0:C], OP.add)
                        bsh = vitpool.tile([128, CW], F32, tag="bsh")
                        nc.vector.tensor_reduce(
                            bsh[:], cand_v, op=OP.max, axis=AX.X)
                        # score(t) = best + e built fully in PSUM: e seeded by
                        # an identity mm (start=True), then 4 gather mms
                        # accumulate the distributed best (single group).
                        # The chain reads score straight from PSUM.
                        psc = pspool.tile([128, 64], F32, tag="lps0",
                                          name="vfps")
                        nc.tensor.matmul(psc[:, 0:CP], ident128[:], e_sl,
                                         start=True, stop=False,
                                         skip_group_check=True)
                        for g in range(4):
                            nc.tensor.matmul(
                                psc[:, g * CW:(g + 1) * CW],
                                gatherG[:, g * 128:(g + 1) * 128], bsh[:],
                                start=False, stop=(g == 3),
                                skip_group_check=True)
                        # history copy (Act, off-chain), deferred one step so
                        # tile-granular tracking can't stall the next cand
                        if pend_copy is not None:
                            nc.scalar.activation(pend_copy[0], pend_copy[1],
                                                 AF.Copy)
                            pend_copy = None
                        if k == 0 and bk >= 1:
                            nc.sync.dma_start(
                                sd[t0 - 4:t0].rearrange("t p c -> p t c"),
                                scoreR[bk - 1][:].rearrange(
                                    "p (s c) -> p s c", c=CP))
                        pend_copy = (out_sl, psc[:, 0:CP])
                        prev_ps = psc
                    if bk == nblk - 1:
                        if pend_copy is not None:
                            nc.scalar.activation(pend_copy[0], pend_copy[1],
                                                 AF.Copy)
                            pend_copy = None
                        nc.sync.dma_start(
                            sd[t0:t0 + 4].rearrange("t p c -> p t c"),
                            scoreR[bk][:].rearrange("p (s c) -> p s c", c=CP))
                    if bk >= 2:
                        del scoreR[bk - 2], et[bk - 2]
            # ---------- backtrace ----------            # ---------- backtrace ----------            # ---------- backtrace ----------
            def backtrace():
                # Single 32-wide chain: per step 1 PE mm (pu = vT @ transT) +
                # add + max-reduce + is_ge + PE transpose + Act copy (vT).
                sd = dt['score_d'][:, :, :]
                ohd = dt['oh_d'][:, :, :]
                BK = 8
                vT = None
                for bk in range(T // BK - 1, -1, -1):
                    t0 = bk * BK   # block covers t0 .. t0+7
                    sc = vitpool.tile([32, BK * CP], F32, tag="sc", name="sc")
                    nc.sync.dma_start(
                        sc[:].rearrange("p (s c) -> p s c", c=CP),
                        sd[t0:t0 + BK].rearrange(
                            "t (b q) c -> b t q c", q=4)[:, :, 0, :])
                    ohr = vitpool.tile([32, BK * CP], F32, tag="ohr", name="ohr")
                    ur = vitpool.tile([32, BK * CP], F32, tag="ur", name="ur")
                    for k in range(BK - 1, -1, -1):
                        t = t0 + k
                        sc_sl = sc[:, k * CP:(k + 1) * CP]
                        u_sl = ur[:, k * CP:(k + 1) * CP]
                        m2 = vitpool.tile([32, 1], F32, tag="m2", name="m2")
                        if t == T - 1:
                            nc.vector.tensor_tensor(u_sl, sc_sl, endRep[:],
                                                    OP.add)
                        else:
                            pub = pspool.tile([64, 64], F32, tag="bt",
                                              name="pub", bufs=1)
                            pu = pub[0:32, 0:CP]
                            nc.tensor.matmul(pu, vT[:], transT[:],
                                             start=True, stop=True)
                            nc.vector.tensor_tensor(u_sl, sc_sl, pu, OP.add)
                        nc.vector.tensor_reduce(m2[:], u_sl, op=OP.max,
                                                axis=AX.X)
                        oh_sl = ohr[:, k * CP:(k + 1) * CP]
                        nc.vector.tensor_tensor(
                            oh_sl, u_sl, m2[:].broadcast_to([32, CP]),
                            OP.is_ge)
                        if t > 0:
                            pT = pspool.tile([64, 64], F32, tag="bt",
                                             name="pT", bufs=1)
                            nc.tensor.transpose(pT[0:CP, 0:32], oh_sl, ident32[:])
                            vT = sigpool.tile([C, 32], F32, tag="vT",
                                              name="vT")
                            nc.vector.tensor_copy(vT[:], pT[0:C, 0:32])
                    nc.sync.dma_start(
                        ohd[t0:t0 + BK, :, :].rearrange("t p c -> p t c"),
                        ohr[:].rearrange("p (s c) -> p s c", c=CP))
            # ---------- extract tags ----------            # ---------- extract tags ----------            # ---------- extract tags ----------
            def extract():
                ohd = dt['oh_d'][:, :, :].rearrange("t b c -> (t b) c")
                NJT = T // 4
                JC = min(16, NJT)
                for jc in range(NJT // JC):
                    ohch = vitpool.tile([128, JC * CP], F32, tag="ohch", bufs=2)
                    nc.sync.dma_start(
                        ohch[:].rearrange("p (j c) -> p j c", c=CP),
                        ohd[:, :].rearrange("(j p) c -> p j c", p=128)[
                            :, jc * JC:(jc + 1) * JC, :])
                    prod = vitpool.tile([128, JC * CP], F32, tag="prod", bufs=2)
                    nc.vector.tensor_tensor(
                        prod[:].rearrange("p (j c) -> p j c", c=CP),
                        ohch[:].rearrange("p (j c) -> p j c", c=CP),
                        iotaRep[:].unsqueeze(1).broadcast_to([128, JC, CP]),
                        OP.mult)
                    tf = vitpool.tile([128, JC], F32, tag="tf")
                    nc.vector.tensor_reduce(
                        tf[:], prod[:].rearrange("p (j c) -> p j c", c=CP),
                        op=OP.add, axis=AX.X)
                    ti = vitpool.tile([128, JC], I32, tag="ti")
                    nc.vector.tensor_copy(ti[:], tf[:])
                    for tl in range(4):
                        nc.sync.dma_start(
                            dt['tags'][:, :].rearrange("b (j f) -> b j f", f=4)[
                                :, jc * JC:(jc + 1) * JC, tl],
                            ti[tl * 32:(tl + 1) * 32, :])

            def next_layer_sched(gen_for_chunk):
                """Feed one chunk per block from block 66: chunk 31-j at
                block 66+2j, chunk 32+j at 67+2j (respects hbuf dump order:
                chunk c readable once dumps of 8t-blocks <= max(2c+1,127-2c)
                are emitted, i.e. from block max(2c+2, 128-2c))."""
                s = {}
                for j in range(31):
                    s.setdefault(66 + 2 * j, []).append(gen_for_chunk(31 - j))
                    s.setdefault(67 + 2 * j, []).append(gen_for_chunk(32 + j))
                # chunks 0 and 63 drain after the loop (post-drain)
                s.setdefault(10 ** 6, [])
                return s, [gen_for_chunk(0), gen_for_chunk(63)]

            ph = 63
            if ph == 63:
                bulk_load_weights(0)
                bulk_load_weights(1)
                bulk_load_weights(2)
                # bulk0: pairs (j, 63-j); j=0,1 fully up front, rest paced
                s0 = {}
                for j in range(2):
                    for ch in (j, 63 - j):
                        for kind, op in bulk_chunk_ops(0, dt['xT'], 40, ch):
                            op()
                for j in range(2, 32):
                    s0.setdefault(2 * (j - 2), []).extend([
                        bulk_chunk_ops(0, dt['xT'], 40, j),
                        bulk_chunk_ops(0, dt['xT'], 40, 63 - j)])
                # bulk1 into L0's tail
                sb1, tail1 = next_layer_sched(
                    lambda c: bulk_chunk_ops(1, dt['hbuf0'], 128, c))
                for blk, gens in sb1.items():
                    s0.setdefault(blk, []).extend(gens)
                lstm_layer(0, dt['hbuf0'], s0, lb=4)
                for g in tail1:
                    for kind, op in g:
                        op()
                # bulk2 into L1
                sb2, tail2 = next_layer_sched(
                    lambda c: bulk_chunk_ops(2, dt['hbuf1'], 128, c))
                lstm_layer(1, dt['hbuf1'], sb2, lb=4)
                for g in tail2:
                    for kind, op in g:
                        op()
                # emissions into L2 (2 chunks/block from block 66)
                se = {}
                for j in range(31):
                    se.setdefault(66 + 2 * j, []).append(
                        emissions_chunk_ops(dt['hbuf2'], 31 - j))
                    se.setdefault(67 + 2 * j, []).append(
                        emissions_chunk_ops(dt['hbuf2'], 32 + j))
                tail_e = [emissions_chunk_ops(dt['hbuf2'], 0),
                          emissions_chunk_ops(dt['hbuf2'], 63)]
                lstm_layer(2, dt['hbuf2'], se, lb=7)
                for g in tail_e:
                    for kind, op in g:
                        op()
                viterbi_fwd()
                backtrace()
                extract()
            else:
                if ph & 1:
                    bulk_gx(0, dt['xT'], 40, BF16)
                if ph & 2:
                    lstm_layer(0, dt['hbuf0'])
                    if ph & 1:
                        bulk_gx(1, dt['hbuf0'], 128, BF16)
                    lstm_layer(1, dt['hbuf1'])
                    if ph & 1:
                        bulk_gx(2, dt['hbuf1'], 128, BF16)
                    lstm_layer(2, dt['hbuf2'])
                if ph & 4:
                    emissions(dt['hbuf2'])
                if ph & 8:
                    viterbi_fwd()
                if ph & 16:
                    backtrace()
                if ph & 32:
                    extract()

    legalize_waits(nc)
    return nc


def make_in_map(inputs, cid, T, wd):
    m = {'xT': shard_x(inputs['x'], cid, T)}
    m.update(wd)
    return m


_CACHE = {}


def kernel(x, w_ih_l0, w_hh_l0, b_l0, w_ih_r, w_hh_r, b_r,
           lin_w, lin_b, crf_start, crf_end, crf_trans):
    """Full BiLSTM-CRF on 8 NeuronCores, data-parallel over the batch."""
    from concourse.bass_utils import run_bass_kernel_spmd
    T = 1024
    if 'nc' not in _CACHE:
        _CACHE['nc'] = build_nc(T)
    nc = _CACHE['nc']
    wd = prep_weights(w_ih_l0, w_hh_l0, b_l0, w_ih_r, w_hh_r, b_r,
                      lin_w, lin_b, crf_start, crf_end, crf_trans)
    in_maps = []
    for cid in range(NCORES):
        m = {'xT': shard_x(x, cid, T)}
        m.update(wd)
        in_maps.append(m)
    res = run_bass_kernel_spmd(nc, in_maps, core_ids=list(range(NCORES)))
    tags = np.concatenate([res.results[c]['tags'] for c in range(NCORES)], axis=0)
    return np.ascontiguousarray(tags.astype(np.int32))



# revision 62
# speedup vs baseline: 1.0182x; 1.0083x over previous
"""Development version of the full-device BiLSTM-CRF kernel. See design notes.

Layouts (per core, BL=32 sequences):
 - LSTM gate-major: partitions = [fwd feat 64; bwd feat 64]; psum free =
   (pair-parity, gate, batch32). Two 16-seq groups pipeline the step chain.
 - gx bulk-matmul'd (f32r/bf16, N=512) into DRAM per direction; identity
   matmul accumulates into PSUM per step pair.
 - Viterbi forward: cp sharded 4-way across partition groups; score/e/onehot
   histories time-folded [128, T/4 * 41] (partition group = t%4).
 - Backtrace: onehot chain via PE matmul with trans^T, TTR fused add+max.
"""
import sys
sys.path.insert(0, '/opt/trn_rl_repo')
import numpy as np
import ml_dtypes
import concourse.bass as bass
import concourse.mybir as mybir
from concourse.tile import TileContext

F32 = mybir.dt.float32
F32R = mybir.dt.float32  # f32r reverted: interp models f32r with reduced precision
BF16 = mybir.dt.float32  # precision experiment: all-f32
I32 = mybir.dt.int32
AF = mybir.ActivationFunctionType
OP = mybir.AluOpType
AX = mybir.AxisListType

B, D_IN, HID, C = 256, 39, 128, 41
H = HID // 2
G4 = 4 * H
NCORES = 8
BL = B // NCORES
CP = 44
NG = 4
CW = 11
NEG = -1.0e30


def legalize_waits(nc):
    n = 0
    for _, bbw in nc.bb_map.items():
        il = bbw.bb.instructions
        out = []
        for i in il:
            si = getattr(i, 'sync_info', None)
            ow = list(si.on_wait) if (si is not None and si.on_wait) else []
            if len(ow) > 1:
                for w in ow[:-1]:
                    n += 1
                    es = mybir.InstEventSemaphore(
                        name=f"legwait-{n}-{i.name}", engine=i.engine, ins=[], outs=[],
                        sync_info=mybir.SyncInfo(on_wait=[w], on_update=[]))
                    out.append(es)
                i.sync_info = mybir.SyncInfo(on_wait=[ow[-1]], on_update=list(si.on_update or []))
            out.append(i)
        bbw.bb.instructions = out
    return n


def prep_weights(w_ih_l0, w_hh_l0, b_l0, w_ih_r, w_hh_r, b_r,
                 lin_w, lin_b, crf_start, crf_end, crf_trans):
    """Gate order i,f,g,o. g rows scaled x2 (tanh(z) = 2*sigmoid(2z)-1)."""
    d = {}

    def gscale(m):
        m = np.asarray(m, np.float32).copy()
        m[2 * H:3 * H] *= 2.0
        return m

    for di, nm in ((0, 'f'), (1, 'b')):
        w = gscale(w_ih_l0[di])
        bb = gscale(b_l0[di])
        d[f'wx0_{nm}'] = np.concatenate([w.T, bb[None, :]], 0).astype(np.float32)
    for li in (0, 1):
        for di, nm in ((0, 'f'), (1, 'b')):
            w = gscale(w_ih_r[li, di])
            bb = gscale(b_r[li, di])
            d[f'wx{li+1}_{nm}'] = np.ascontiguousarray(w.T).astype(np.float32)
            d[f'bias{li+1}_{nm}'] = bb[None, :].astype(np.float32)
    for li in range(3):
        whh = np.asarray(w_hh_l0) if li == 0 else np.asarray(w_hh_r[li - 1])
        for gi in range(4):
            blk = np.zeros((128, 128), np.float32)
            sc = 2.0 if gi == 2 else 1.0
            blk[0:64, 0:64] = sc * whh[0, gi * H:(gi + 1) * H, :].T
            blk[64:128, 64:128] = sc * whh[1, gi * H:(gi + 1) * H, :].T
            d[f'whh{li}_{gi}'] = blk.astype(np.float32)
    d['ident128'] = np.eye(128, dtype=np.float32)
    d['ident16'] = np.eye(16, dtype=np.float32)
    d['ident32'] = np.eye(32, dtype=np.float32)
    d['ident44'] = np.eye(CP, dtype=np.float32)
    lw = np.zeros((HID, CP), np.float32)
    lw[:, :C] = np.asarray(lin_w, np.float32).T
    d['linWT'] = lw.astype(np.float32)
    lb = np.full((CP, 1), NEG, np.float32)
    lb[:C, 0] = np.asarray(lin_b, np.float32)
    d['linB'] = lb
    tr = np.asarray(crf_trans, np.float32)
    # transB_cn[p=(b,g), (ci, cp)] = trans[cp, g*CW+ci], NEG for pads
    transB = np.full((128, CW, CP), NEG, np.float32)
    for g in range(NG):
        for ci in range(CW):
            cn = g * CW + ci
            if cn < C:
                for b in range(32):
                    transB[b * 4 + g, ci, :C] = tr[:, cn]
    d['transB'] = transB.reshape(128, CW * CP)
    trT = np.full((C, CP), NEG, np.float32)
    trT[:, :C] = tr.T  # [cn, cp]
    d['transT'] = trT
    d['transThi'] = np.ascontiguousarray(trT[32:41])  # cn 32..40 rows
    st = np.full((128, CP), NEG, np.float32)
    st[:, :C] = np.asarray(crf_start, np.float32)[None, :]
    d['startRep'] = st
    en = np.full((32, CP), NEG, np.float32)
    en[:, :C] = np.asarray(crf_end, np.float32)[None, :]
    d['endRep'] = en
    io = np.zeros((128, CP), np.float32)
    io[:, :C] = np.arange(C, dtype=np.float32)[None, :]
    d['iotaRep'] = io
    d['onesrow'] = np.ones((1, 512), np.float32)
    gG = np.zeros((128, 4, 128), np.float32)
    for b in range(32):
        for g in range(4):
            for gp in range(4):
                gG[b * 4 + g, g, b * 4 + gp] = 1.0
    d['gatherG'] = gG.reshape(128, 512)
    d['zeros16'] = np.zeros((128, 16), np.float32)
    return d


def shard_x(x, cid, T):
    xs = np.asarray(x, np.float32)[cid * BL:(cid + 1) * BL, :T]
    xt = np.empty((D_IN + 1, T * BL), np.float32)
    xt[D_IN] = 1.0
    xt[:D_IN] = xs.transpose(2, 1, 0).reshape(D_IN, T * BL)
    return xt.astype(np.float32)


def build_nc(T):
    R = BL * T
    TJ = T // 4
    NCH = R // 512
    nc = bass.Bass()
    dt = {}

    def din(name, shape, dty=F32):
        dt[name] = nc.dram_tensor(name, shape, dty, kind="ExternalInput")

    din('xT', [D_IN + 1, R], F32R)
    din('wx0_f', [40, 256], F32R); din('wx0_b', [40, 256], F32R)
    for li in (1, 2):
        for nm in ('f', 'b'):
            din(f'wx{li}_{nm}', [128, 256], F32R)
            din(f'bias{li}_{nm}', [1, 256], F32R)
    for li in range(3):
        for gi in range(4):
            din(f'whh{li}_{gi}', [128, 128], F32)
    din('ident128', [128, 128], BF16); din('ident16', [16, 16]); din('ident32', [32, 32]); din('ident44', [CP, CP])
    din('linWT', [HID, CP], F32R); din('linB', [CP, 1])
    din('transB', [128, CW * CP]); din('transT', [C, CP])
    din('transThi', [9, CP])
    din('startRep', [128, CP]); din('endRep', [32, CP]); din('iotaRep', [128, CP])
    din('onesrow', [1, 512], F32R); din('zeros16', [128, 16], F32)
    din('gatherG', [128, 512])

    def scratch(name, shape, dty=F32):
        dt[name] = nc.dram_tensor(name, shape, dty, kind="Internal")

    for li3 in range(3):
        scratch(f'gx{li3}_f', [64, T * 128], BF16)
        scratch(f'gx{li3}_b', [64, T * 128], BF16)
    scratch('hbuf0', [HID, R], F32R)
    scratch('hbuf1', [HID, R], F32R)
    scratch('hbuf2', [HID, R], F32R)
    scratch('e_d', [T, 128, CP])
    scratch('score_d', [T, 128, CP])
    scratch('oh_d', [T, 32, CP])
    dt['tags'] = nc.dram_tensor('tags', [BL, T], I32, kind="ExternalOutput")

    with TileContext(nc) as tc:
        with tc.tile_pool(name="const", bufs=1) as cpool, \
             tc.tile_pool(name="wpool", bufs=1) as wpool, \
             tc.tile_pool(name="hist", bufs=1) as hpool, \
             tc.tile_pool(name="bulk_rhs", bufs=4) as rhspool, \
             tc.tile_pool(name="gx", bufs=8) as gxpool, \
             tc.tile_pool(name="psum", bufs=2, space="PSUM") as pspool, \
             tc.tile_pool(name="sig", bufs=8) as sigpool, \
             tc.tile_pool(name="hc", bufs=8) as hcpool, \
             tc.tile_pool(name="vit", bufs=4) as vitpool, \
             tc.tile_pool(name="emis", bufs=2) as epool:

            def load_const(nm, shape, dty=F32):
                t = cpool.tile(shape, dty, tag=nm)
                nc.sync.dma_start(t[:], dt[nm][:])
                return t

            ident128 = load_const('ident128', [128, 128], BF16)
            ident16 = load_const('ident16', [16, 16])
            ident32 = load_const('ident32', [32, 32])
            ident44 = load_const('ident44', [CP, CP])
            linWT = load_const('linWT', [HID, CP], F32R)
            linB = load_const('linB', [CP, 1])
            transB = load_const('transB', [128, CW * CP])
            transT = load_const('transT', [C, CP])
            transThi = load_const('transThi', [9, CP])
            startRep = load_const('startRep', [128, CP])
            endRep = load_const('endRep', [32, CP])
            iotaRep = load_const('iotaRep', [128, CP])
            gatherG = load_const('gatherG', [128, 512])
            whh = {}
            for li in range(3):
                for gi in range(4):
                    whh[(li, gi)] = load_const(f'whh{li}_{gi}', [128, 128], F32)
            onesrow = load_const('onesrow', [1, 512], F32R)
            zeros16 = load_const('zeros16', [128, 16], F32)

            scoreRep = hpool.tile([128, CP], F32, tag="scoreRep")

            # ---------- bulk gx (micro-op generator for interleaving) ----------
            bulk_state = {}

            def bulk_load_weights(li):
                wx = {}
                bias = {}
                for nm in ('f', 'b'):
                    wx[nm] = wpool.tile([40 if li == 0 else 128, 256], F32R,
                                        tag=f"wx{li}_{nm}", name=f"wx{li}{nm}")
                    nc.sync.dma_start(wx[nm][:], dt[f'wx{li}_{nm}'][:])
                    if li > 0:
                        bias[nm] = wpool.tile([1, 256], F32R, tag=f"bias{li}_{nm}", name=f"bias{li}{nm}")
                        nc.sync.dma_start(bias[nm][:], dt[f'bias{li}_{nm}'][:])
                bulk_state[li] = (wx, bias)

            def bulk_chunk_ops(li, src_dram, src_k, ch):
                """Yield micro-closures; caller drains them spread over time."""
                wx, bias = bulk_state[li]
                rhs = rhspool.tile([src_k, 512], F32R, tag=f"rhs{li}")
                yield ('l', lambda: nc.sync.dma_start(
                    rhs[:], src_dram[:, ch * 512:(ch + 1) * 512]))
                for nm in ('f', 'b'):
                    for pr in range(2):
                        ps = pspool.tile([128, 512], F32, tag="big", name="bps")
                        # matmuls split into 256-col halves so each drained
                        # piece fits the PE idle window of one LSTM step
                        for hh in range(4):
                            sl = slice(hh * 128, (hh + 1) * 128)
                            if li == 0:
                                yield ('h', lambda ps=ps, nm=nm, pr=pr, sl=sl:
                                       nc.tensor.matmul(
                                    ps[:, sl], wx[nm][:, pr * 128:(pr + 1) * 128],
                                    rhs[:, sl], start=True, stop=True,
                                    skip_group_check=True))
                            else:
                                yield ('h', lambda ps=ps, nm=nm, pr=pr, sl=sl:
                                       nc.tensor.matmul(
                                    ps[:, sl], wx[nm][:, pr * 128:(pr + 1) * 128],
                                    rhs[:, sl], start=True, stop=False,
                                    skip_group_check=True))
                                yield ('h', lambda ps=ps, nm=nm, pr=pr, sl=sl:
                                       nc.tensor.matmul(
                                    ps[:, sl], bias[nm][:, pr * 128:(pr + 1) * 128],
                                    onesrow[:, sl], start=False, stop=True,
                                    skip_group_check=True))
                        stg = rhspool.tile([128, 512], BF16, tag="gxstg",
                                           name="gxstg")
                        if (ch + pr) % 2 == 0:
                            yield ('l', lambda ps=ps, stg=stg: nc.scalar.activation(
                                stg[:], ps[:], AF.Copy))
                        else:
                            yield ('l', lambda ps=ps, stg=stg: nc.vector.tensor_copy(
                                stg[:], ps[:]))
                        gxd = dt[f'gx{li}_{nm}']
                        t0c = ch * 16
                        for gl in range(2):
                            gi4 = pr * 2 + gl
                            yield ('l', lambda stg=stg, gxd=gxd, t0c=t0c, gi4=gi4, gl=gl: \
                                nc.sync.dma_start(
                                    gxd[:, :].rearrange("p (t g b) -> p t g b",
                                                        g=4, b=32)[
                                        :, t0c:t0c + 16, gi4, :],
                                    stg[gl * 64:(gl + 1) * 64, :].rearrange(
                                        "p (t b) -> p t b", b=32)))

            def bulk_gx(li, src_dram, src_k, rhs_dty):
                bulk_load_weights(li)
                for ch in range(NCH):
                    for kind, op in bulk_chunk_ops(li, src_dram, src_k, ch):
                        op()

            # ---------- LSTM recurrence (skewed dual-chain pipeline) ----------
            def lstm_layer(li, hbuf_out, sched=None, lb=3):
                """sched: dict block_idx -> list of micro-op generators; ops
                drain in order, <=1 heavy + <=lb light per step, at iter end."""
                from collections import deque
                pending = deque()

                def drain(hb, lb):
                    while pending:
                        kind, fn = pending[0]
                        if kind == 'h':
                            if hb <= 0:
                                break
                            hb -= 1
                        else:
                            if lb <= 0:
                                break
                            lb -= 1
                        pending.popleft()
                        fn()

                gxf, gxb = dt[f'gx{li}_f'], dt[f'gx{li}_b']
                NB = T // 8
                cts = {}
                hprev = {}
                for g2 in range(2):
                    cts[g2] = hcpool.tile([128, 16], F32, tag=f"c{g2}", name=f"c{g2}")
                    nc.vector.memset(cts[g2][:], 0.0)
                    hprev[g2] = zeros16
                gxt = {}
                hring = {}
                pss = {}
                sig = {}
                ths = {}

                def load_block(blk):
                    t0 = blk * 8
                    g = gxpool.tile([128, 8 * 128], BF16, tag="gx")
                    nc.sync.dma_start(g[0:64, :], gxf[:, t0 * 128:(t0 + 8) * 128])
                    # bwd: reversed-t read so slot k holds t = T-1-t0-k
                    nc.sync.dma_start(
                        g[64:128, :].rearrange("p (s f) -> p s f", f=128),
                        gxb[:, :].rearrange("p (t f) -> p t f", f=128)[
                            :, T - 1 - t0:T - 9 - t0 if T - 9 - t0 >= 0 else None:-1, :])
                    gxt[blk] = g
                    hring[blk] = {
                        g2: hcpool.tile([128, 8 * 16], F32, tag=f"hr{g2}",
                                        name=f"hr{g2}") for g2 in range(2)}

                def dump_block(blk):
                    t0 = blk * 8
                    for g2 in range(2):
                        bs = g2 * 16
                        hr = hring[blk][g2]
                        nc.sync.dma_start(
                            hbuf_out[0:64, :].rearrange("p (t b) -> p t b", b=BL)[
                                :, t0:t0 + 8, bs:bs + 16],
                            hr[0:64, :].rearrange("p (s b) -> p s b", b=16))
                        nc.sync.dma_start(
                            hbuf_out[64:128, :].rearrange("p (t b) -> p t b", b=BL)[
                                :, T - 1 - t0:T - 9 - t0 if T - 9 - t0 >= 0 else None:-1,
                                bs:bs + 16],
                            hr[64:128, :].rearrange("p (s b) -> p s b", b=16))
                    del gxt[blk], hring[blk]

                def S1(g2, k):     # PE: inject gx + accumulate whh gates
                    blk, kk = divmod(k, 8)
                    bs = g2 * 16
                    ps = pspool.tile([128, 64], F32, tag=f"lps{g2}",
                                     name=f"lps{g2}")
                    gxt_v = gxt[blk][:].rearrange("p (s g b) -> p s g b",
                                                  g=4, b=32)
                    nc.tensor.matmul(
                        ps[:].rearrange("p (g b) -> p g b", g=4),
                        ident128[:], gxt_v[:, kk, :, bs:bs + 16],
                        start=True, stop=False)
                    for gi in range(4):
                        nc.tensor.matmul(
                            ps[:, gi * 16:(gi + 1) * 16],
                            whh[(li, gi)][:], hprev[g2][:],
                            start=False, stop=(gi == 3), skip_group_check=True)
                    pss[g2] = ps

                def S2(g2, k):     # Act: all-gate sigmoid
                    s = sigpool.tile([128, 64], F32, tag=f"sig{g2}",
                                     name=f"sig{g2}")
                    nc.scalar.activation(s[:], pss[g2][:], AF.Sigmoid)
                    sig[g2] = s

                def S3(g2, k):     # DVE+Pool: cell-state update
                    s = sig[g2]
                    A = sigpool.tile([128, 16], F32, tag=f"A{g2}", name=f"A{g2}")
                    nc.vector.tensor_tensor(A[:], s[:, 0:16], s[:, 32:48],
                                            OP.mult)
                    Bt = sigpool.tile([128, 16], F32, tag=f"B{g2}", name=f"B{g2}")
                    nc.vector.scalar_tensor_tensor(Bt[:], A[:], 2.0, s[:, 0:16],
                                                   OP.mult, OP.subtract)
                    Ct = sigpool.tile([128, 16], F32, tag=f"C{g2}", name=f"C{g2}")
                    nc.gpsimd.tensor_tensor(Ct[:], s[:, 16:32], cts[g2][:],
                                            OP.mult)
                    cn = hcpool.tile([128, 16], F32, tag=f"c{g2}", name=f"c{g2}")
                    nc.vector.tensor_tensor(cn[:], Bt[:], Ct[:], OP.add)
                    cts[g2] = cn

                def S4(g2, k):     # Act: tanh(c)
                    th = sigpool.tile([128, 16], F32, tag=f"th{g2}",
                                      name=f"th{g2}")
                    nc.scalar.activation(th[:], cts[g2][:], AF.Tanh)
                    ths[g2] = th

                def S5(g2, k):     # DVE: h = o * tanh(c) into ring slot
                    blk, kk = divmod(k, 8)
                    hn = hring[blk][g2][:, kk * 16:(kk + 1) * 16]
                    nc.vector.tensor_tensor(hn, sig[g2][:, 48:64], ths[g2][:],
                                            OP.mult)
                    hprev[g2] = hn

                for k in range(T):
                    blk, kk = divmod(k, 8)
                    if kk == 0:
                        if sched:
                            for gen in sched.get(blk, []):
                                pending.extend(gen)
                        load_block(blk)
                    S1(0, k)
                    if k > 0:
                        S4(1, k - 1)
                        S5(1, k - 1)
                        if kk == 0:
                            dump_block(blk - 1)
                    S2(0, k)
                    drain(1, lb // 3)
                    S1(1, k)
                    S3(0, k)
                    S2(1, k)
                    drain(2, lb // 3)
                    S4(0, k)
                    S3(1, k)
                    S5(0, k)
                    drain(1, lb - 2 * (lb // 3))
                S4(1, T - 1)
                S5(1, T - 1)
                dump_block(NB - 1)
                drain(10 ** 9, 10 ** 9)

            # ---------- emissions (micro-op generator) ----------
            def emissions_chunk_ops(hsrc, ch):
                rhs = rhspool.tile([128, 512], F32R, tag="erhs")
                yield ('l', lambda: nc.sync.dma_start(
                    rhs[:], hsrc[:, ch * 512:(ch + 1) * 512]))
                psb = pspool.tile([128, 512], F32, tag="big", name="epsb")
                ps = psb[0:CP, :]
                for hh in range(4):
                    sl = slice(hh * 128, (hh + 1) * 128)
                    yield ('h', lambda sl=sl: nc.tensor.matmul(
                        ps[:, sl], linWT[:], rhs[:, sl],
                        start=True, stop=True, skip_group_check=True))
                eo = epool.tile([CP, 512], F32, tag="eo")
                yield ('l', lambda: nc.scalar.activation(eo[:], ps, AF.Identity,
                                                         bias=linB[:]))
                for k in range(4):
                    psTb = pspool.tile([128, 64], F32, tag="epsT", name="psTb", bufs=1)
                    psT = psTb[:, 0:CP]
                    yield ('l', lambda psT=psT, k=k: nc.tensor.transpose(
                        psT, eo[:, k * 128:(k + 1) * 128], ident44[:]))
                    estg = epool.tile([128, CP], F32, tag="estg")
                    if k % 2 == 0:
                        yield ('l', lambda psT=psT, estg=estg: \
                               nc.scalar.activation(estg[:], psT, AF.Copy))
                    else:
                        yield ('l', lambda psT=psT, estg=estg: \
                               nc.vector.tensor_copy(estg[:], psT))
                    tb = ch * 4 + k
                    ed = dt['e_d'][:, :, :].rearrange("t p c -> (t p) c")
                    yield ('l', lambda estg=estg, tb=tb, ed=ed: \
                           nc.sync.dma_start(
                        ed[:, :].rearrange("(t b q) c -> (t b) q c", b=32, q=4)[
                            tb * 128:(tb + 1) * 128, :, :],
                        estg[:].unsqueeze(1).broadcast_to([128, 4, CP])))

            def emissions(hsrc):
                for ch in range(NCH):
                    for kind, op in emissions_chunk_ops(hsrc, ch):
                        op()
            # ---------- viterbi forward ----------            # ---------- viterbi forward ----------
            def viterbi_fwd():
                ed = dt['e_d'][:, :, :]
                sd = dt['score_d'][:, :, :]
                transB_v = transB[:].rearrange("p (w c) -> p w c", c=CP)
                scoreR = {}
                et = {}
                pend_copy = None
                prev_ps = None
                nblk = T // 4
                for bk in range(nblk):
                    t0 = bk * 4
                    et[bk] = vitpool.tile([128, 4 * CP], F32, tag="ein", name="ein")
                    nc.sync.dma_start(
                        et[bk][:].rearrange("p (s c) -> p s c", c=CP),
                        ed[t0:t0 + 4].rearrange("t p c -> p t c"))
                    scoreR[bk] = vitpool.tile([128, 4 * CP], F32, tag="sring",
                                              name="sring")
                    for k in range(4):
                        t = t0 + k
                        e_sl = et[bk][:, k * CP:(k + 1) * CP]
                        out_sl = scoreR[bk][:, k * CP:(k + 1) * CP]
                        if t == 0:
                            nc.vector.tensor_tensor(out_sl, startRep[:], e_sl, OP.add)
                            continue
                        prev = scoreR[bk][:, 0:CP] if t == 1 else \
                            prev_ps[:, 0:CP]
                        cand = vitpool.tile([128, CW * C], F32, tag="cand")
                        cand_v = cand[:].rearrange("p (w c) -> p w c", c=C)
                        nc.vector.tensor_tensor(
                            cand_v,
                            prev[:, 0:C].unsqueeze(1).broadcast_to([128, CW, C]),
                            transB_v[:, :, 0:C], OP.add)
                        bsh = vitpool.tile([128, CW], F32, tag="bsh")
                        nc.vector.tensor_reduce(
                            bsh[:], cand_v, op=OP.max, axis=AX.X)
                        # score(t) = best + e built fully in PSUM: e seeded by
                        # an identity mm (start=True), then 4 gather mms
                        # accumulate the distributed best (single group).
                        # The chain reads score straight from PSUM.
                        psc = pspool.tile([128, 64], F32, tag="lps0",
                                          name="vfps")
                        nc.tensor.matmul(psc[:, 0:CP], ident128[:], e_sl,
                                         start=True, stop=False,
                                         skip_group_check=True)
                        for g in range(4):
                            nc.tensor.matmul(
                                psc[:, g * CW:(g + 1) * CW],
                                gatherG[:, g * 128:(g + 1) * 128], bsh[:],
                                start=False, stop=(g == 3),
                                skip_group_check=True)
                        # history copy (Act, off-chain), deferred one step so
                        # tile-granular tracking can't stall the next cand
                        if pend_copy is not None:
                            nc.scalar.activation(pend_copy[0], pend_copy[1],
                                                 AF.Copy)
                            pend_copy = None
                        if k == 0 and bk >= 1:
                            nc.sync.dma_start(
                                sd[t0 - 4:t0].rearrange("t p c -> p t c"),
                                scoreR[bk - 1][:].rearrange(
                                    "p (s c) -> p s c", c=CP))
                        pend_copy = (out_sl, psc[:, 0:CP])
                        prev_ps = psc
                    if bk == nblk - 1:
                        if pend_copy is not None:
                            nc.scalar.activation(pend_copy[0], pend_copy[1],
                                                 AF.Copy)
                            pend_copy = None
                        nc.sync.dma_start(
                            sd[t0:t0 + 4].rearrange("t p c -> p t c"),
                            scoreR[bk][:].rearrange("p (s c) -> p s c", c=CP))
                    if bk >= 2:
                        del scoreR[bk - 2], et[bk - 2]
            # ---------- backtrace ----------            # ---------- backtrace ----------            # ---------- backtrace ----------
            def backtrace():
                # Single 32-wide chain: per step 1 PE mm (pu = vT @ transT) +
                # add + max-reduce + is_ge + PE transpose + Act copy (vT).
                sd = dt['score_d'][:, :, :]
                ohd = dt['oh_d'][:, :, :]
                BK = 8
                vT = None
                for bk in range(T // BK - 1, -1, -1):
                    t0 = bk * BK   # block covers t0 .. t0+7
                    sc = vitpool.tile([32, BK * CP], F32, tag="sc", name="sc")
                    nc.sync.dma_start(
                        sc[:].rearrange("p (s c) -> p s c", c=CP),
                        sd[t0:t0 + BK].rearrange(
                            "t (b q) c -> b t q c", q=4)[:, :, 0, :])
                    ohr = vitpool.tile([32, BK * CP], F32, tag="ohr", name="ohr")
                    ur = vitpool.tile([32, BK * CP], F32, tag="ur", name="ur")
                    for k in range(BK - 1, -1, -1):
                        t = t0 + k
                        sc_sl = sc[:, k * CP:(k + 1) * CP]
                        u_sl = ur[:, k * CP:(k + 1) * CP]
                        m2 = vitpool.tile([32, 1], F32, tag="m2", name="m2")
                        if t == T - 1:
                            nc.vector.tensor_tensor(u_sl, sc_sl, endRep[:],
                                                    OP.add)
                        else:
                            pub = pspool.tile([64, 64], F32, tag="bt",
                                              name="pub", bufs=1)
                            pu = pub[0:32, 0:CP]
                            nc.tensor.matmul(pu, vT[:], transT[:],
                                             start=True, stop=True)
                            nc.vector.tensor_tensor(u_sl, sc_sl, pu, OP.add)
                        nc.vector.tensor_reduce(m2[:], u_sl, op=OP.max,
                                                axis=AX.X)
                        oh_sl = ohr[:, k * CP:(k + 1) * CP]
                        nc.vector.tensor_tensor(
                            oh_sl, u_sl, m2[:].broadcast_to([32, CP]),
                            OP.is_ge)
                        if t > 0:
                            pT = pspool.tile([64, 64], F32, tag="bt",
                                             name="pT", bufs=1)
                            nc.tensor.transpose(pT[0:CP, 0:32], oh_sl, ident32[:])
                            vT = sigpool.tile([C, 32], F32, tag="vT",
                                              name="vT")
                            nc.vector.tensor_copy(vT[:], pT[0:C, 0:32])
                    nc.sync.dma_start(
                        ohd[t0:t0 + BK, :, :].rearrange("t p c -> p t c"),
                        ohr[:].rearrange("p (s c) -> p s c", c=CP))
            # ---------- extract tags ----------            # ---------- extract tags ----------            # ---------- extract tags ----------
            def extract():
                ohd = dt['oh_d'][:, :, :].rearrange("t b c -> (t b) c")
                NJT = T // 4
                JC = min(16, NJT)
                for jc in range(NJT // JC):
                    ohch = vitpool.tile([128, JC * CP], F32, tag="ohch", bufs=2)
                    nc.sync.dma_start(
                        ohch[:].rearrange("p (j c) -> p j c", c=CP),
                        ohd[:, :].rearrange("(j p) c -> p j c", p=128)[
                            :, jc * JC:(jc + 1) * JC, :])
                    prod = vitpool.tile([128, JC * CP], F32, tag="prod", bufs=2)
                    nc.vector.tensor_tensor(
                        prod[:].rearrange("p (j c) -> p j c", c=CP),
                        ohch[:].rearrange("p (j c) -> p j c", c=CP),
                        iotaRep[:].unsqueeze(1).broadcast_to([128, JC, CP]),
                        OP.mult)
                    tf = vitpool.tile([128, JC], F32, tag="tf")
                    nc.vector.tensor_reduce(
                        tf[:], prod[:].rearrange("p (j c) -> p j c", c=CP),
                        op=OP.add, axis=AX.X)
                    ti = vitpool.tile([128, JC], I32, tag="ti")
                    nc.vector.tensor_copy(ti[:], tf[:])
                    for tl in range(4):
                        nc.sync.dma_start(
                            dt['tags'][:, :].rearrange("b (j f) -> b j f", f=4)[
                                :, jc * JC:(jc + 1) * JC, tl],
                            ti[tl * 32:(tl + 1) * 32, :])

            def next_layer_sched(gen_for_chunk):
                """Feed one chunk per block from block 66: chunk 31-j at
                block 66+2j, chunk 32+j at 67+2j (respects hbuf dump order:
                chunk c readable once dumps of 8t-blocks <= max(2c+1,127-2c)
                are emitted, i.e. from block max(2c+2, 128-2c))."""
                s = {}
                for j in range(31):
                    s.setdefault(66 + 2 * j, []).append(gen_for_chunk(31 - j))
                    s.setdefault(67 + 2 * j, []).append(gen_for_chunk(32 + j))
                # chunks 0 and 63 drain after the loop (post-drain)
                s.setdefault(10 ** 6, [])
                return s, [gen_for_chunk(0), gen_for_chunk(63)]

            ph = 63
            if ph == 63:
                bulk_load_weights(0)
                bulk_load_weights(1)
                bulk_load_weights(2)
                # bulk0: pairs (j, 63-j); j=0,1 fully up front, rest paced
                s0 = {}
                for j in range(2):
                    for ch in (j, 63 - j):
                        for kind, op in bulk_chunk_ops(0, dt['xT'], 40, ch):
                            op()
                for j in range(2, 32):
                    s0.setdefault(2 * (j - 2), []).extend([
                        bulk_chunk_ops(0, dt['xT'], 40, j),
                        bulk_chunk_ops(0, dt['xT'], 40, 63 - j)])
                # bulk1 into L0's tail
                sb1, tail1 = next_layer_sched(
                    lambda c: bulk_chunk_ops(1, dt['hbuf0'], 128, c))
                for blk, gens in sb1.items():
                    s0.setdefault(blk, []).extend(gens)
                lstm_layer(0, dt['hbuf0'], s0, lb=4)
                for g in tail1:
                    for kind, op in g:
                        op()
                # bulk2 into L1
                sb2, tail2 = next_layer_sched(
                    lambda c: bulk_chunk_ops(2, dt['hbuf1'], 128, c))
                lstm_layer(1, dt['hbuf1'], sb2, lb=4)
                for g in tail2:
                    for kind, op in g:
                        op()
                # emissions into L2 (2 chunks/block from block 66)
                se = {}
                for j in range(31):
                    se.setdefault(66 + 2 * j, []).append(
                        emissions_chunk_ops(dt['hbuf2'], 31 - j))
                    se.setdefault(67 + 2 * j, []).append(
                        emissions_chunk_ops(dt['hbuf2'], 32 + j))
                tail_e = [emissions_chunk_ops(dt['hbuf2'], 0),
                          emissions_chunk_ops(dt['hbuf2'], 63)]
                lstm_layer(2, dt['hbuf2'], se, lb=7)
                for g in tail_e:
                    for kind, op in g:
                        op()
                viterbi_fwd()
                backtrace()
                extract()
            else:
                if ph & 1:
                    bulk_gx(0, dt['xT'], 40, BF16)
                if ph & 2:
                    lstm_layer(0, dt['hbuf0'])
                    if ph & 1:
                        bulk_gx(1, dt['hbuf0'], 128, BF16)
                    lstm_layer(1, dt['hbuf1'])
                    if ph & 1:
                        bulk_gx(2, dt['hbuf1'], 128, BF16)
                    lstm_layer(2, dt['hbuf2'])
                if ph & 4:
                    emissions(dt['hbuf2'])
                if ph & 8:
                    viterbi_fwd()
                if ph & 16:
                    backtrace()
                if ph & 32:
                    extract()

    legalize_waits(nc)
    return nc


def make_in_map(inputs, cid, T, wd):
    m = {'xT': shard_x(inputs['x'], cid, T)}
    m.update(wd)
    return m


_CACHE = {}


def kernel(x, w_ih_l0, w_hh_l0, b_l0, w_ih_r, w_hh_r, b_r,
           lin_w, lin_b, crf_start, crf_end, crf_trans):
    """Full BiLSTM-CRF on 8 NeuronCores, data-parallel over the batch."""
    from concourse.bass_utils import run_bass_kernel_spmd
    T = 1024
    if 'nc' not in _CACHE:
        _CACHE['nc'] = build_nc(T)
    nc = _CACHE['nc']
    wd = prep_weights(w_ih_l0, w_hh_l0, b_l0, w_ih_r, w_hh_r, b_r,
                      lin_w, lin_b, crf_start, crf_end, crf_trans)
    in_maps = []
    for cid in range(NCORES):
        m = {'xT': shard_x(x, cid, T)}
        m.update(wd)
        in_maps.append(m)
    res = run_bass_kernel_spmd(nc, in_maps, core_ids=list(range(NCORES)))
    tags = np.concatenate([res.results[c]['tags'] for c in range(NCORES)], axis=0)
    return np.ascontiguousarray(tags.astype(np.int32))

